# revision 13
# baseline (speedup 1.0000x reference)
"""MultiHeadAttention forward on 8 trn2 NeuronCores (Bass/Tile).

Problem: B=2, S=2048, D=1024, H=16, DK=64.  reference returns (out, attn):
  Q = q @ Wq.T ; K = k @ Wk.T ; V = v @ Wv.T   (per-head split)
  S = Q K^T / sqrt(DK) ; A = softmax(S) ; ctx = A V ; out = ctx @ Wo.T + b_o

Sharding: data-parallel over B (2) x tensor-parallel over heads (4 groups of
4 heads) = 8 cores.  Each core computes its 4 heads' attention entirely in the
transposed orientation (S^T with k on partitions), which gives softmax row
sums for free via a ones-column appended to V during the PV matmul, and needs
only ONE exp pass on the scalar engine.  The normalized attention is staged
TRANSPOSED ([head, k, q]) in bf16; the host transposes/upcasts during the
unshard step.  Output projection partial products (one per head group) are
summed on the host during the gather.

All GEMMs run in bf16 with fp32 PSUM accumulation.  Scores matmuls pack the
two heads of a pair into disjoint PE row groups (K=64 each) so they run
concurrently.  PV/attention work for one q-half is interleaved into the next
q-half's scores stream to keep the PE dense.
"""

import os
import sys

sys.path.insert(0, "/opt/trn_rl_repo")

from contextlib import ExitStack

import ml_dtypes
import numpy as np

B, S, D, H = 2, 2048, 1024, 16
DK = D // H  # 64
N_CORES = 8
HG = H // (N_CORES // B)  # heads per core = 4
NPAIR = HG // 2  # head pairs per core = 2
MT = D // 128  # 8 m-tiles (contraction tiles for projections)
KT = S // 128  # 16 k-tiles
QC = S // 512  # 4 q-chunks

_CACHED = {}


def _build_program():
    import concourse.bass as bass
    import concourse.tile as tile
    from concourse import bacc, mybir

    BF16 = mybir.dt.bfloat16
    F32 = mybir.dt.float32
    AF = mybir.ActivationFunctionType

    nc = bacc.Bacc(trn_type="TRN2", target_bir_lowering=False, debug=False)

    # ---- DRAM I/O (per-core shapes) ----
    xqT = nc.dram_tensor("xqT", [D, S], BF16, kind="ExternalInput").ap()
    xkT = nc.dram_tensor("xkT", [D, S], BF16, kind="ExternalInput").ap()
    xvT = nc.dram_tensor("xvT", [D, S], BF16, kind="ExternalInput").ap()
    wqT = nc.dram_tensor("wqT", [D, 2 * 128], BF16, kind="ExternalInput").ap()
    wkT = nc.dram_tensor("wkT", [D, 2 * 128], BF16, kind="ExternalInput").ap()
    wvT = nc.dram_tensor("wvT", [D, 2 * 128], BF16, kind="ExternalInput").ap()
    woT = nc.dram_tensor("woT", [2 * 128, D], BF16, kind="ExternalInput").ap()
    attnT = nc.dram_tensor("attnT", [HG, S, S], BF16, kind="ExternalOutput").ap()
    outp = nc.dram_tensor("outp", [S, D], F32, kind="ExternalOutput").ap()

    scale = float(1.0 / np.sqrt(np.float32(DK)))

    with tile.TileContext(nc, pool_alloc_mode="queue") as tc, ExitStack() as ctx:
        # preload the ACT table set containing BOTH Exp and Ln (and Copy) so
        # the auto-inserted table loads never ping-pong between sets.
        # act_func_set_id 6 = natural_log_exp_and_others (act_info.json).
        nc.scalar.add_instruction(
            mybir.InstLoadActFuncSet(
                name=nc.get_next_instruction_name(),
                ins=[],
                outs=[],
                act_func_set_id=6,
            )
        )

        # ---------- persistent pools ----------
        qk_pool = ctx.enter_context(tc.tile_pool(name="qk", bufs=4))
        vp_pool = ctx.enter_context(tc.tile_pool(name="vp", bufs=HG * KT))
        wo_pool = ctx.enter_context(tc.tile_pool(name="wo", bufs=2))
        ctx_pool = ctx.enter_context(tc.tile_pool(name="ctxT", bufs=2))
        rz_pool = ctx.enter_context(tc.tile_pool(name="rz", bufs=3))
        sm_pool = ctx.enter_context(tc.tile_pool(name="sm", bufs=4))
        stage_pool = ctx.enter_context(tc.tile_pool(name="stage", bufs=3))
        osb_pool = ctx.enter_context(tc.tile_pool(name="osb", bufs=2))

        ps_small = ctx.enter_context(
            tc.tile_pool(name="ps_small", bufs=2, space="PSUM")
        )

        # persistent SBUF tensors
        QT2 = [qk_pool.tile([128, S], BF16, tag="qk", name=f"QT2_{p}") for p in range(NPAIR)]
        KT2 = [qk_pool.tile([128, S], BF16, tag="qk", name=f"KT2_{p}") for p in range(NPAIR)]
        Vp = [
            [vp_pool.tile([128, DK + 1], BF16, tag="vp", name=f"Vp_{h}_{kt}") for kt in range(KT)]
            for h in range(HG)
        ]
        woT_sb = [wo_pool.tile([128, D], BF16, tag="wo", name=f"woT_sb_{i}") for i in range(2)]
        ctxT2 = [ctx_pool.tile([128, S], BF16, tag="ctxT", name=f"ctxT2_{p}") for p in range(NPAIR)]

        for i in range(2):
            nc.sync.dma_start(woT_sb[i][:], woT[128 * i : 128 * (i + 1), :])

        def load_w_tiles(wT, nm, pool):
            tiles = []
            for mt in range(MT):
                t = pool.tile([128, 256], BF16, tag="wt", name=f"wt_{nm}_{mt}")
                nc.sync.dma_start(t[:], wT[128 * mt : 128 * (mt + 1), :])
                tiles.append(t)
            return tiles

        def load_x_tiles(xT, nm, pool):
            tiles = []
            for mt in range(MT):
                t = pool.tile([128, S], BF16, tag="xt", name=f"xt_{nm}_{mt}")
                nc.sync.dma_start(t[:], xT[128 * mt : 128 * (mt + 1), :])
                tiles.append(t)
            return tiles

        # ---------- phase 0: Q^T / K^T projections ----------
        with ExitStack() as p0:
            xt_pool = p0.enter_context(tc.tile_pool(name="xt", bufs=16))
            wt_pool = p0.enter_context(tc.tile_pool(name="wt", bufs=16))

            # weights first (small, needed by the first matmuls), then the
            # big input loads so DMA runs ahead
            wtq = load_w_tiles(wqT, "q", wt_pool)
            wtk = load_w_tiles(wkT, "k", wt_pool)
            xtq = load_x_tiles(xqT, "q", xt_pool)
            xtk = load_x_tiles(xkT, "k", xt_pool)

            # --- Q^T and K^T: out[do, s] = sum_m wT[m, do] * xT[m, s] ---
            for xt, wt, DEST in ((xtq, wtq, QT2), (xtk, wtk, KT2)):
                for p in range(NPAIR):
                    for c in range(QC):
                        ps = ps_small.tile([128, 512], F32, tag="ps_small")
                        for mt in range(MT):
                            nc.tensor.matmul(
                                ps[:],
                                wt[mt][:, 128 * p : 128 * (p + 1)],
                                xt[mt][:, 512 * c : 512 * (c + 1)],
                                start=(mt == 0),
                                stop=(mt == MT - 1),
                            )
                        nc.vector.tensor_copy(
                            DEST[p][:, 512 * c : 512 * (c + 1)], ps[:]
                        )

        def v_block(st):
            # one s-tile of the V projection: out[s, dv] = sum_m xvT * wvT
            ps = ps_small.tile([128, 256], F32, tag="ps_small", name=f"vps_{st}")
            for mt in range(MT):
                nc.tensor.matmul(
                    ps[:],
                    xtv[mt][:, 128 * st : 128 * (st + 1)],
                    wtv[mt][:],
                    start=(mt == 0),
                    stop=(mt == MT - 1),
                )
            for h in range(HG):
                nc.vector.tensor_copy(
                    Vp[h][st][:, 0:DK], ps[:, DK * h : DK * (h + 1)]
                )
                nc.vector.memset(Vp[h][st][:, DK : DK + 1], 1.0)

        # ---------- attention ----------
        with ExitStack() as pa:
            pt_pool_a = pa.enter_context(tc.tile_pool(name="pt_a", bufs=40))
            ps_s = pa.enter_context(tc.tile_pool(name="ps_s", bufs=3, space="PSUM"))
            pt_pools = [pt_pool_a]
            pv0 = ExitStack()
            xv_pool = pv0.enter_context(tc.tile_pool(name="xv", bufs=8))
            wv_pool = pv0.enter_context(tc.tile_pool(name="wv", bufs=8))
            wtv = load_w_tiles(wvT, "v", wv_pool)
            xtv = load_x_tiles(xvT, "v", xv_pool)

            # PT[h][kt][qh]: exp(S^T) bf16 half tiles [128 k, 1024 q]
            PT = [[[None] * 2 for _ in range(KT)] for _ in range(HG)]
            rzrep = {}  # (h, qh) -> [128, 1024] bf16 recipZ replicated

            def pv_job_a(p, hh, c):
                """PV accumulation for chunk c of head 2p+hh, plus Z ->
                recipZ on ACT.  Returns state for pv_job_b."""
                h = 2 * p + hh
                qh, sub = divmod(c, 2)
                pv = ps_small.tile(
                    [DK + 1, 512], F32, tag="ps_small", name=f"pv_{h}_{c}"
                )
                for kt in range(KT):
                    nc.tensor.matmul(
                        pv[:],
                        Vp[h][kt][:],
                        PT[h][kt][qh][:, 512 * sub : 512 * (sub + 1)],
                        start=(kt == 0),
                        stop=(kt == KT - 1),
                    )
                lnz = sm_pool.tile([1, 512], F32, tag="sm", name=f"lnz_{h}_{c}")
                nc.scalar.activation(lnz[:], pv[DK : DK + 1, :], AF.Ln)
                rz = sm_pool.tile([1, 512], F32, tag="sm", name=f"rz_{h}_{c}")
                nc.scalar.activation(rz[:], lnz[:], AF.Exp, scale=-1.0)
                return (p, hh, c, pv, rz)

            def pv_job_b(state):
                """Broadcast recipZ across partitions, normalize ctx^T,
                build bf16 recipZ replica for the attention staging."""
                p, hh, c, pv, rz = state
                h = 2 * p + hh
                qh, sub = divmod(c, 2)
                zrf = sm_pool.tile([128, 512], F32, tag="zrf", name=f"zrf_{h}_{c}")
                nc.gpsimd.partition_broadcast(zrf[:], rz[:])
                nc.vector.tensor_mul(
                    ctxT2[p][64 * hh : 64 * (hh + 1), 512 * c : 512 * (c + 1)],
                    pv[0:DK, :],
                    zrf[0:DK, :],
                )
                key = (h, qh)
                if key not in rzrep:
                    rzrep[key] = rz_pool.tile(
                        [128, 1024], BF16, tag="rz", name=f"rzrep_{h}_{qh}"
                    )
                nc.vector.tensor_copy(
                    rzrep[key][:, 512 * sub : 512 * (sub + 1)], zrf[:]
                )

            def attn_stage(h, qh):
                """Normalize P~^T and DMA the staged transposed attention.
                Alternate DVE / GpSimd so the two elementwise engines split
                the multiply work."""
                for kt in range(KT):
                    at = stage_pool.tile(
                        [128, 1024], BF16, tag="stage", name=f"at_{h}_{kt}_{qh}"
                    )
                    nc.vector.tensor_mul(at[:], PT[h][kt][qh][:], rzrep[(h, qh)][:])
                    nc.sync.dma_start(
                        attnT[
                            h,
                            128 * kt : 128 * (kt + 1),
                            1024 * qh : 1024 * (qh + 1),
                        ],
                        at[:],
                    )

            def scores_phase(p, qh, jobs, fillers=()):
                """S^T + exp for both heads of pair p over q-half qh, with
                the previous q-half's PV/normalize work interleaved.

                jobs: list of (p_prev, hh, c) chunk descriptors.
                fillers: callables emitted (spread out) into the phase."""
                pending_b = None
                pending_stage = None
                points = {1: 0, 4: 1, 7: 2, 10: 3, 13: 4}
                fillers = list(fillers)
                nfill = len(fillers)
                for kt in range(KT):
                    if nfill:
                        lo = nfill * kt // KT
                        hi = nfill * (kt + 1) // KT
                        for fi in range(lo, hi):
                            fillers[fi]()
                    for hh in range(2):
                        h = 2 * p + hh
                        pool = pt_pools[(2 * kt + hh) % len(pt_pools)]
                        ptile = pool.tile(
                            [128, 1024], BF16, tag="pt", name=f"PT_{h}_{kt}_{qh}"
                        )
                        PT[h][kt][qh] = ptile
                        ps = ps_s.tile(
                            [128, 1024], F32, tag="ps_s", name=f"st_{h}_{kt}_{qh}"
                        )
                        for sub in range(2):
                            q0 = 1024 * qh + 512 * sub
                            nc.tensor.matmul(
                                ps[:, 512 * sub : 512 * (sub + 1)],
                                KT2[p][
                                    64 * hh : 64 * (hh + 1),
                                    128 * kt : 128 * (kt + 1),
                                ],
                                QT2[p][64 * hh : 64 * (hh + 1), q0 : q0 + 512],
                                start=True,
                                stop=True,
                            )
                        nc.scalar.activation(ptile[:], ps[:], AF.Exp, scale=scale)
                    if kt in points:
                        j = points[kt]
                        if pending_stage is not None:
                            attn_stage(*pending_stage)
                            pending_stage = None
                        if pending_b is not None:
                            pv_job_b(pending_b)
                            jp, jhh, jc = jobs[j - 1]
                            if jc % 2 == 1:  # second chunk of this head done
                                pending_stage = (2 * jp + jhh, jc // 2)
                            pending_b = None
                        if j < len(jobs):
                            pending_b = pv_job_a(*jobs[j])
                # drain leftovers (the j=4 point only handles job 3's b-half)
                if pending_b is not None:
                    pv_job_b(pending_b)
                    jp, jhh, jc = jobs[-1]
                    if jc % 2 == 1:
                        pending_stage = (2 * jp + jhh, jc // 2)
                if pending_stage is not None:
                    attn_stage(*pending_stage)

            def jobs_for(p, qh):
                return [
                    (p, 0, 2 * qh + 0),
                    (p, 0, 2 * qh + 1),
                    (p, 1, 2 * qh + 0),
                    (p, 1, 2 * qh + 1),
                ]

            def outproj(qts):
                for qt in qts:
                    for mc in range(2):
                        ps = ps_small.tile(
                            [128, 512], F32, tag="ps_small", name=f"ops_{qt}_{mc}"
                        )
                        for p in range(NPAIR):
                            nc.tensor.matmul(
                                ps[:],
                                ctxT2[p][:, 128 * qt : 128 * (qt + 1)],
                                woT_sb[p][:, 512 * mc : 512 * (mc + 1)],
                                start=(p == 0),
                                stop=(p == NPAIR - 1),
                            )
                        osb = osb_pool.tile(
                            [128, 512], F32, tag="osb", name=f"osb_{qt}_{mc}"
                        )
                        nc.scalar.copy(osb[:], ps[:])
                        nc.sync.dma_start(
                            outp[
                                128 * qt : 128 * (qt + 1),
                                512 * mc : 512 * (mc + 1),
                            ],
                            osb[:],
                        )

            import functools

            scores_phase(
                0, 0, [], fillers=[functools.partial(v_block, st) for st in range(KT)]
            )
            pv0.close()  # xv/wv freed; grow the P~^T pool into that space
            pt_pool_b = pa.enter_context(tc.tile_pool(name="pt_b", bufs=18))
            pt_pools.append(pt_pool_b)
            scores_phase(0, 1, jobs_for(0, 0))
            scores_phase(1, 0, jobs_for(0, 1))
            scores_phase(1, 1, jobs_for(1, 0))
            # drain: PV for the last q-half, interleaved with the output
            # projection for the q-tiles whose ctx columns are already final
            # (chunks 0-1 = q-tiles 0-7 completed during the last phase).
            jobs = jobs_for(1, 1)
            st0 = pv_job_a(*jobs[0])
            st1 = pv_job_a(*jobs[1])
            pv_job_b(st0)
            outproj(range(0, 3))
            pv_job_b(st1)
            st2 = pv_job_a(*jobs[2])
            attn_stage(2 * jobs[1][0] + jobs[1][1], 1)
            outproj(range(3, 6))
            st3 = pv_job_a(*jobs[3])
            pv_job_b(st2)
            outproj(range(6, 8))
            pv_job_b(st3)
            attn_stage(2 * jobs[3][0] + jobs[3][1], 1)
            outproj(range(8, KT))

    nc.compile()
    return nc


def _get_program():
    if "nc" not in _CACHED:
        _CACHED["nc"] = _build_program()
    return _CACHED["nc"]


def _reference_numpy(query, key, value, mask, w_q, w_k, w_v, w_o, b_o):
    """Fallback for inputs the fast path does not handle (masked rows)."""
    scale = np.sqrt(np.float32(DK))
    q = np.asarray(query, np.float32)
    k = np.asarray(key, np.float32)
    v = np.asarray(value, np.float32)
    Q = (q @ np.asarray(w_q, np.float32).T).reshape(B, S, H, DK).transpose(0, 2, 1, 3)
    K = (k @ np.asarray(w_k, np.float32).T).reshape(B, S, H, DK).transpose(0, 2, 1, 3)
    V = (v @ np.asarray(w_v, np.float32).T).reshape(B, S, H, DK).transpose(0, 2, 1, 3)
    scores = np.einsum("bhqd,bhkd->bhqk", Q, K) / scale
    m = np.asarray(mask)[:, None, None, :]
    scores = np.where(m == 0, np.float32(-1e9), scores)
    scores -= scores.max(-1, keepdims=True)
    e = np.exp(scores)
    attn = (e / e.sum(-1, keepdims=True)).astype(np.float32)
    ctx = np.einsum("bhqk,bhkd->bhqd", attn, V)
    ctx = ctx.transpose(0, 2, 1, 3).reshape(B, S, D)
    out = (ctx @ np.asarray(w_o, np.float32).T + np.asarray(b_o, np.float32)).astype(
        np.float32
    )
    return out, attn


def kernel(query, key, value, mask, w_q, w_k, w_v, w_o, b_o):
    from concourse.bass_utils import run_bass_kernel_spmd

    query = np.asarray(query, np.float32)
    key = np.asarray(key, np.float32)
    value = np.asarray(value, np.float32)
    mask_np = np.asarray(mask)
    if not np.all(mask_np == 1):
        return _reference_numpy(query, key, value, mask_np, w_q, w_k, w_v, w_o, b_o)
    w_q = np.asarray(w_q, np.float32)
    w_k = np.asarray(w_k, np.float32)
    w_v = np.asarray(w_v, np.float32)
    w_o = np.asarray(w_o, np.float32)
    b_o = np.asarray(b_o, np.float32)

    nc = _get_program()

    bf = ml_dtypes.bfloat16
    in_maps = []
    for c in range(N_CORES):
        b, g = divmod(c, N_CORES // B)
        hs = slice(256 * g, 256 * (g + 1))
        in_maps.append(
            {
                "xqT": np.ascontiguousarray(query[b].T).astype(bf),
                "xkT": np.ascontiguousarray(key[b].T).astype(bf),
                "xvT": np.ascontiguousarray(value[b].T).astype(bf),
                "wqT": np.ascontiguousarray(w_q[hs, :].T).astype(bf),
                "wkT": np.ascontiguousarray(w_k[hs, :].T).astype(bf),
                "wvT": np.ascontiguousarray(w_v[hs, :].T).astype(bf),
                "woT": np.ascontiguousarray(w_o[:, hs].T).astype(bf),
            }
        )

    trace = bool(int(os.environ.get("TRN_TRACE", "0")))
    res = run_bass_kernel_spmd(nc, in_maps, list(range(N_CORES)), trace=trace)
    if trace and res.exec_time_ns is not None:
        print(f"HW exec time: {res.exec_time_ns} ns")
        _CACHED["exec_time_ns"] = res.exec_time_ns
        _CACHED["trace"] = res.instructions_and_trace

    out = np.empty((B, S, D), np.float32)
    attn = np.empty((B, H, S, S), np.float32)
    partials = np.zeros((B, S, D), np.float32)
    for c in range(N_CORES):
        b, g = divmod(c, N_CORES // B)
        r = res.results[c]
        partials[b] += r["outp"]
        a = r["attnT"]  # [HG, S(k), S(q)] bf16
        for j in range(HG):
            attn[b, HG * g + j] = a[j].T.astype(np.float32)
    for b in range(B):
        out[b] = partials[b] + b_o
    return out, attn


# revision 14
# speedup vs baseline: 1.0486x; 1.0486x over previous
"""MultiHeadAttention forward on 8 trn2 NeuronCores (Bass/Tile).

Problem: B=2, S=2048, D=1024, H=16, DK=64.  reference returns (out, attn):
  Q = q @ Wq.T ; K = k @ Wk.T ; V = v @ Wv.T   (per-head split)
  S = Q K^T / sqrt(DK) ; A = softmax(S) ; ctx = A V ; out = ctx @ Wo.T + b_o

Sharding: data-parallel over B (2) x tensor-parallel over heads (4 groups of
4 heads) = 8 cores.  Each core computes its 4 heads' attention entirely in the
transposed orientation (S^T with k on partitions), which gives softmax row
sums for free via a ones-column appended to V during the PV matmul, and needs
only ONE exp pass on the scalar engine.  The normalized attention is staged
TRANSPOSED ([head, k, q]) in bf16; the host transposes/upcasts during the
unshard step.  Output projection partial products (one per head group) are
summed on the host during the gather.

All GEMMs run in bf16 with fp32 PSUM accumulation.  Scores matmuls pack the
two heads of a pair into disjoint PE row groups (K=64 each) so they run
concurrently.  PV/attention work for one q-half is interleaved into the next
q-half's scores stream to keep the PE dense.
"""

import os
import sys

sys.path.insert(0, "/opt/trn_rl_repo")

from contextlib import ExitStack

import ml_dtypes
import numpy as np

B, S, D, H = 2, 2048, 1024, 16
DK = D // H  # 64
N_CORES = 8
HG = H // (N_CORES // B)  # heads per core = 4
NPAIR = HG // 2  # head pairs per core = 2
MT = D // 128  # 8 m-tiles (contraction tiles for projections)
KT = S // 128  # 16 k-tiles
QC = S // 512  # 4 q-chunks

_CACHED = {}


def _build_program():
    import concourse.bass as bass
    import concourse.tile as tile
    from concourse import bacc, mybir

    BF16 = mybir.dt.bfloat16
    F32 = mybir.dt.float32
    AF = mybir.ActivationFunctionType

    nc = bacc.Bacc(trn_type="TRN2", target_bir_lowering=False, debug=False)

    # ---- DRAM I/O (per-core shapes) ----
    xqT = nc.dram_tensor("xqT", [D, S], BF16, kind="ExternalInput").ap()
    xkT = nc.dram_tensor("xkT", [D, S], BF16, kind="ExternalInput").ap()
    xvT = nc.dram_tensor("xvT", [D, S], BF16, kind="ExternalInput").ap()
    wqT = nc.dram_tensor("wqT", [D, 2 * 128], BF16, kind="ExternalInput").ap()
    wkT = nc.dram_tensor("wkT", [D, 2 * 128], BF16, kind="ExternalInput").ap()
    wvT = nc.dram_tensor("wvT", [D, 2 * 128], BF16, kind="ExternalInput").ap()
    woT = nc.dram_tensor("woT", [2 * 128, D], BF16, kind="ExternalInput").ap()
    attnT = nc.dram_tensor("attnT", [HG, S, S], BF16, kind="ExternalOutput").ap()
    outp = nc.dram_tensor("outp", [S, D], F32, kind="ExternalOutput").ap()

    scale = float(1.0 / np.sqrt(np.float32(DK)))

    with tile.TileContext(nc) as tc, ExitStack() as ctx:
        # preload the ACT table set containing BOTH Exp and Ln (and Copy) so
        # the auto-inserted table loads never ping-pong between sets.
        # act_func_set_id 6 = natural_log_exp_and_others (act_info.json).
        nc.scalar.add_instruction(
            mybir.InstLoadActFuncSet(
                name=nc.get_next_instruction_name(),
                ins=[],
                outs=[],
                act_func_set_id=6,
            )
        )

        # ---------- persistent pools ----------
        qk_pool = ctx.enter_context(tc.tile_pool(name="qk", bufs=4))
        vp_pool = ctx.enter_context(tc.tile_pool(name="vp", bufs=HG * KT))
        wo_pool = ctx.enter_context(tc.tile_pool(name="wo", bufs=2))
        ctx_pool = ctx.enter_context(tc.tile_pool(name="ctxT", bufs=2))
        rz_pool = ctx.enter_context(tc.tile_pool(name="rz", bufs=3))
        sm_pool = ctx.enter_context(tc.tile_pool(name="sm", bufs=4))
        stage_pool = ctx.enter_context(tc.tile_pool(name="stage", bufs=3))
        osb_pool = ctx.enter_context(tc.tile_pool(name="osb", bufs=2))

        ps_small = ctx.enter_context(
            tc.tile_pool(name="ps_small", bufs=2, space="PSUM")
        )

        # persistent SBUF tensors
        QT2 = [qk_pool.tile([128, S], BF16, tag="qk", name=f"QT2_{p}") for p in range(NPAIR)]
        KT2 = [qk_pool.tile([128, S], BF16, tag="qk", name=f"KT2_{p}") for p in range(NPAIR)]
        Vp = [
            [vp_pool.tile([128, DK + 1], BF16, tag="vp", name=f"Vp_{h}_{kt}") for kt in range(KT)]
            for h in range(HG)
        ]
        woT_sb = [wo_pool.tile([128, D], BF16, tag="wo", name=f"woT_sb_{i}") for i in range(2)]
        ctxT2 = [ctx_pool.tile([128, S], BF16, tag="ctxT", name=f"ctxT2_{p}") for p in range(NPAIR)]

        for i in range(2):
            nc.sync.dma_start(woT_sb[i][:], woT[128 * i : 128 * (i + 1), :])

        def load_w_tiles(wT, nm, pool):
            tiles = []
            for mt in range(MT):
                t = pool.tile([128, 256], BF16, tag="wt", name=f"wt_{nm}_{mt}")
                nc.sync.dma_start(t[:], wT[128 * mt : 128 * (mt + 1), :])
                tiles.append(t)
            return tiles

        def load_x_tiles(xT, nm, pool):
            tiles = []
            for mt in range(MT):
                t = pool.tile([128, S], BF16, tag="xt", name=f"xt_{nm}_{mt}")
                nc.sync.dma_start(t[:], xT[128 * mt : 128 * (mt + 1), :])
                tiles.append(t)
            return tiles

        # ---------- phase 0: Q^T / K^T projections ----------
        with ExitStack() as p0:
            xt_pool = p0.enter_context(tc.tile_pool(name="xt", bufs=16))
            wt_pool = p0.enter_context(tc.tile_pool(name="wt", bufs=16))

            # weights first (small, needed by the first matmuls), then the
            # big input loads so DMA runs ahead
            wtq = load_w_tiles(wqT, "q", wt_pool)
            wtk = load_w_tiles(wkT, "k", wt_pool)
            xtq = load_x_tiles(xqT, "q", xt_pool)
            xtk = load_x_tiles(xkT, "k", xt_pool)

            # --- Q^T and K^T: out[do, s] = sum_m wT[m, do] * xT[m, s] ---
            for xt, wt, DEST in ((xtq, wtq, QT2), (xtk, wtk, KT2)):
                for p in range(NPAIR):
                    for c in range(QC):
                        ps = ps_small.tile([128, 512], F32, tag="ps_small")
                        for mt in range(MT):
                            nc.tensor.matmul(
                                ps[:],
                                wt[mt][:, 128 * p : 128 * (p + 1)],
                                xt[mt][:, 512 * c : 512 * (c + 1)],
                                start=(mt == 0),
                                stop=(mt == MT - 1),
                            )
                        nc.vector.tensor_copy(
                            DEST[p][:, 512 * c : 512 * (c + 1)], ps[:]
                        )

        def v_block(st):
            # one s-tile of the V projection: out[s, dv] = sum_m xvT * wvT
            ps = ps_small.tile([128, 256], F32, tag="ps_small", name=f"vps_{st}")
            for mt in range(MT):
                nc.tensor.matmul(
                    ps[:],
                    xtv[mt][:, 128 * st : 128 * (st + 1)],
                    wtv[mt][:],
                    start=(mt == 0),
                    stop=(mt == MT - 1),
                )
            for h in range(HG):
                nc.vector.tensor_copy(
                    Vp[h][st][:, 0:DK], ps[:, DK * h : DK * (h + 1)]
                )
                nc.vector.memset(Vp[h][st][:, DK : DK + 1], 1.0)

        # ---------- attention ----------
        with ExitStack() as pa:
            pt_pool_a = pa.enter_context(tc.tile_pool(name="pt_a", bufs=40))
            ps_s = pa.enter_context(tc.tile_pool(name="ps_s", bufs=3, space="PSUM"))
            pt_pools = [pt_pool_a]
            pv0 = ExitStack()
            xv_pool = pv0.enter_context(tc.tile_pool(name="xv", bufs=8))
            wv_pool = pv0.enter_context(tc.tile_pool(name="wv", bufs=8))
            wtv = load_w_tiles(wvT, "v", wv_pool)
            xtv = load_x_tiles(xvT, "v", xv_pool)

            # PT[h][kt][qh]: exp(S^T) bf16 half tiles [128 k, 1024 q]
            PT = [[[None] * 2 for _ in range(KT)] for _ in range(HG)]
            rzrep = {}  # (h, qh) -> [128, 1024] bf16 recipZ replicated

            def pv_job_a(p, hh, c):
                """PV accumulation for chunk c of head 2p+hh, plus Z ->
                recipZ on ACT.  Returns state for pv_job_b."""
                h = 2 * p + hh
                qh, sub = divmod(c, 2)
                pv = ps_small.tile(
                    [DK + 1, 512], F32, tag="ps_small", name=f"pv_{h}_{c}"
                )
                for kt in range(KT):
                    nc.tensor.matmul(
                        pv[:],
                        Vp[h][kt][:],
                        PT[h][kt][qh][:, 512 * sub : 512 * (sub + 1)],
                        start=(kt == 0),
                        stop=(kt == KT - 1),
                    )
                lnz = sm_pool.tile([1, 512], F32, tag="sm", name=f"lnz_{h}_{c}")
                nc.scalar.activation(lnz[:], pv[DK : DK + 1, :], AF.Ln)
                rz = sm_pool.tile([1, 512], F32, tag="sm", name=f"rz_{h}_{c}")
                nc.scalar.activation(rz[:], lnz[:], AF.Exp, scale=-1.0)
                return (p, hh, c, pv, rz)

            def pv_job_b(state):
                """Broadcast recipZ across partitions, normalize ctx^T,
                build bf16 recipZ replica for the attention staging."""
                p, hh, c, pv, rz = state
                h = 2 * p + hh
                qh, sub = divmod(c, 2)
                zrf = sm_pool.tile([128, 512], F32, tag="zrf", name=f"zrf_{h}_{c}")
                nc.gpsimd.partition_broadcast(zrf[:], rz[:])
                nc.vector.tensor_mul(
                    ctxT2[p][64 * hh : 64 * (hh + 1), 512 * c : 512 * (c + 1)],
                    pv[0:DK, :],
                    zrf[0:DK, :],
                )
                key = (h, qh)
                if key not in rzrep:
                    rzrep[key] = rz_pool.tile(
                        [128, 1024], BF16, tag="rz", name=f"rzrep_{h}_{qh}"
                    )
                nc.vector.tensor_copy(
                    rzrep[key][:, 512 * sub : 512 * (sub + 1)], zrf[:]
                )

            def attn_stage(h, qh):
                """Normalize P~^T and DMA the staged transposed attention.
                Alternate DVE / GpSimd so the two elementwise engines split
                the multiply work."""
                for kt in range(KT):
                    at = stage_pool.tile(
                        [128, 1024], BF16, tag="stage", name=f"at_{h}_{kt}_{qh}"
                    )
                    nc.vector.tensor_mul(at[:], PT[h][kt][qh][:], rzrep[(h, qh)][:])
                    nc.sync.dma_start(
                        attnT[
                            h,
                            128 * kt : 128 * (kt + 1),
                            1024 * qh : 1024 * (qh + 1),
                        ],
                        at[:],
                    )

            def scores_phase(p, qh, jobs, fillers=()):
                """S^T + exp for both heads of pair p over q-half qh, with
                the previous q-half's PV/normalize work interleaved.

                jobs: list of (p_prev, hh, c) chunk descriptors.
                fillers: callables emitted (spread out) into the phase."""
                pending_b = None
                pending_stage = None
                points = {1: 0, 4: 1, 7: 2, 10: 3, 13: 4}
                fillers = list(fillers)
                nfill = len(fillers)
                for kt in range(KT):
                    if nfill:
                        lo = nfill * kt // KT
                        hi = nfill * (kt + 1) // KT
                        for fi in range(lo, hi):
                            fillers[fi]()
                    for hh in range(2):
                        h = 2 * p + hh
                        pool = pt_pools[(2 * kt + hh) % len(pt_pools)]
                        ptile = pool.tile(
                            [128, 1024], BF16, tag="pt", name=f"PT_{h}_{kt}_{qh}"
                        )
                        PT[h][kt][qh] = ptile
                        ps = ps_s.tile(
                            [128, 1024], F32, tag="ps_s", name=f"st_{h}_{kt}_{qh}"
                        )
                        for sub in range(2):
                            q0 = 1024 * qh + 512 * sub
                            nc.tensor.matmul(
                                ps[:, 512 * sub : 512 * (sub + 1)],
                                KT2[p][
                                    64 * hh : 64 * (hh + 1),
                                    128 * kt : 128 * (kt + 1),
                                ],
                                QT2[p][64 * hh : 64 * (hh + 1), q0 : q0 + 512],
                                start=True,
                                stop=True,
                            )
                        nc.scalar.activation(ptile[:], ps[:], AF.Exp, scale=scale)
                    if kt in points:
                        j = points[kt]
                        if pending_stage is not None:
                            attn_stage(*pending_stage)
                            pending_stage = None
                        if pending_b is not None:
                            pv_job_b(pending_b)
                            jp, jhh, jc = jobs[j - 1]
                            if jc % 2 == 1:  # second chunk of this head done
                                pending_stage = (2 * jp + jhh, jc // 2)
                            pending_b = None
                        if j < len(jobs):
                            pending_b = pv_job_a(*jobs[j])
                # drain leftovers (the j=4 point only handles job 3's b-half)
                if pending_b is not None:
                    pv_job_b(pending_b)
                    jp, jhh, jc = jobs[-1]
                    if jc % 2 == 1:
                        pending_stage = (2 * jp + jhh, jc // 2)
                if pending_stage is not None:
                    attn_stage(*pending_stage)

            def jobs_for(p, qh):
                return [
                    (p, 0, 2 * qh + 0),
                    (p, 0, 2 * qh + 1),
                    (p, 1, 2 * qh + 0),
                    (p, 1, 2 * qh + 1),
                ]

            def outproj(qts):
                for qt in qts:
                    for mc in range(2):
                        ps = ps_small.tile(
                            [128, 512], F32, tag="ps_small", name=f"ops_{qt}_{mc}"
                        )
                        for p in range(NPAIR):
                            nc.tensor.matmul(
                                ps[:],
                                ctxT2[p][:, 128 * qt : 128 * (qt + 1)],
                                woT_sb[p][:, 512 * mc : 512 * (mc + 1)],
                                start=(p == 0),
                                stop=(p == NPAIR - 1),
                            )
                        osb = osb_pool.tile(
                            [128, 512], F32, tag="osb", name=f"osb_{qt}_{mc}"
                        )
                        nc.scalar.copy(osb[:], ps[:])
                        nc.sync.dma_start(
                            outp[
                                128 * qt : 128 * (qt + 1),
                                512 * mc : 512 * (mc + 1),
                            ],
                            osb[:],
                        )

            import functools

            scores_phase(
                0, 0, [], fillers=[functools.partial(v_block, st) for st in range(KT)]
            )
            pv0.close()  # xv/wv freed
            scores_phase(0, 1, jobs_for(0, 0))
            scores_phase(1, 0, jobs_for(0, 1))
            scores_phase(1, 1, jobs_for(1, 0))
            # drain: PV for the last q-half, interleaved with the output
            # projection for the q-tiles whose ctx columns are already final
            # (chunks 0-1 = q-tiles 0-7 completed during the last phase).
            jobs = jobs_for(1, 1)
            st0 = pv_job_a(*jobs[0])
            st1 = pv_job_a(*jobs[1])
            pv_job_b(st0)
            outproj(range(0, 3))
            pv_job_b(st1)
            st2 = pv_job_a(*jobs[2])
            attn_stage(2 * jobs[1][0] + jobs[1][1], 1)
            outproj(range(3, 6))
            st3 = pv_job_a(*jobs[3])
            pv_job_b(st2)
            outproj(range(6, 8))
            pv_job_b(st3)
            attn_stage(2 * jobs[3][0] + jobs[3][1], 1)
            outproj(range(8, KT))

    nc.compile()
    return nc


def _get_program():
    if "nc" not in _CACHED:
        _CACHED["nc"] = _build_program()
    return _CACHED["nc"]


def _reference_numpy(query, key, value, mask, w_q, w_k, w_v, w_o, b_o):
    """Fallback for inputs the fast path does not handle (masked rows)."""
    scale = np.sqrt(np.float32(DK))
    q = np.asarray(query, np.float32)
    k = np.asarray(key, np.float32)
    v = np.asarray(value, np.float32)
    Q = (q @ np.asarray(w_q, np.float32).T).reshape(B, S, H, DK).transpose(0, 2, 1, 3)
    K = (k @ np.asarray(w_k, np.float32).T).reshape(B, S, H, DK).transpose(0, 2, 1, 3)
    V = (v @ np.asarray(w_v, np.float32).T).reshape(B, S, H, DK).transpose(0, 2, 1, 3)
    scores = np.einsum("bhqd,bhkd->bhqk", Q, K) / scale
    m = np.asarray(mask)[:, None, None, :]
    scores = np.where(m == 0, np.float32(-1e9), scores)
    scores -= scores.max(-1, keepdims=True)
    e = np.exp(scores)
    attn = (e / e.sum(-1, keepdims=True)).astype(np.float32)
    ctx = np.einsum("bhqk,bhkd->bhqd", attn, V)
    ctx = ctx.transpose(0, 2, 1, 3).reshape(B, S, D)
    out = (ctx @ np.asarray(w_o, np.float32).T + np.asarray(b_o, np.float32)).astype(
        np.float32
    )
    return out, attn


def kernel(query, key, value, mask, w_q, w_k, w_v, w_o, b_o):
    from concourse.bass_utils import run_bass_kernel_spmd

    query = np.asarray(query, np.float32)
    key = np.asarray(key, np.float32)
    value = np.asarray(value, np.float32)
    mask_np = np.asarray(mask)
    if not np.all(mask_np == 1):
        return _reference_numpy(query, key, value, mask_np, w_q, w_k, w_v, w_o, b_o)
    w_q = np.asarray(w_q, np.float32)
    w_k = np.asarray(w_k, np.float32)
    w_v = np.asarray(w_v, np.float32)
    w_o = np.asarray(w_o, np.float32)
    b_o = np.asarray(b_o, np.float32)

    nc = _get_program()

    bf = ml_dtypes.bfloat16
    in_maps = []
    for c in range(N_CORES):
        b, g = divmod(c, N_CORES // B)
        hs = slice(256 * g, 256 * (g + 1))
        in_maps.append(
            {
                "xqT": np.ascontiguousarray(query[b].T).astype(bf),
                "xkT": np.ascontiguousarray(key[b].T).astype(bf),
                "xvT": np.ascontiguousarray(value[b].T).astype(bf),
                "wqT": np.ascontiguousarray(w_q[hs, :].T).astype(bf),
                "wkT": np.ascontiguousarray(w_k[hs, :].T).astype(bf),
                "wvT": np.ascontiguousarray(w_v[hs, :].T).astype(bf),
                "woT": np.ascontiguousarray(w_o[:, hs].T).astype(bf),
            }
        )

    trace = bool(int(os.environ.get("TRN_TRACE", "0")))
    res = run_bass_kernel_spmd(nc, in_maps, list(range(N_CORES)), trace=trace)
    if trace and res.exec_time_ns is not None:
        print(f"HW exec time: {res.exec_time_ns} ns")
        _CACHED["exec_time_ns"] = res.exec_time_ns
        _CACHED["trace"] = res.instructions_and_trace

    out = np.empty((B, S, D), np.float32)
    attn = np.empty((B, H, S, S), np.float32)
    partials = np.zeros((B, S, D), np.float32)
    for c in range(N_CORES):
        b, g = divmod(c, N_CORES // B)
        r = res.results[c]
        partials[b] += r["outp"]
        a = r["attnT"]  # [HG, S(k), S(q)] bf16
        for j in range(HG):
            attn[b, HG * g + j] = a[j].T.astype(np.float32)
    for b in range(B):
        out[b] = partials[b] + b_o
    return out, attn


# revision 15
# speedup vs baseline: 1.0513x; 1.0026x over previous
"""MultiHeadAttention forward on 8 trn2 NeuronCores (Bass/Tile).

Problem: B=2, S=2048, D=1024, H=16, DK=64.  reference returns (out, attn):
  Q = q @ Wq.T ; K = k @ Wk.T ; V = v @ Wv.T   (per-head split)
  S = Q K^T / sqrt(DK) ; A = softmax(S) ; ctx = A V ; out = ctx @ Wo.T + b_o

Sharding: data-parallel over B (2) x tensor-parallel over heads (4 groups of
4 heads) = 8 cores.  Each core computes its 4 heads' attention entirely in the
transposed orientation (S^T with k on partitions), which gives softmax row
sums for free via a ones-column appended to V during the PV matmul, and needs
only ONE exp pass on the scalar engine.  The normalized attention is staged
TRANSPOSED ([head, k, q]) in bf16; the host transposes/upcasts during the
unshard step.  Output projection partial products (one per head group) are
summed on the host during the gather.

All GEMMs run in bf16 with fp32 PSUM accumulation.  Scores matmuls pack the
two heads of a pair into disjoint PE row groups (K=64 each) so they run
concurrently.  PV/attention work for one q-half is interleaved into the next
q-half's scores stream to keep the PE dense.
"""

import os
import sys

sys.path.insert(0, "/opt/trn_rl_repo")

from contextlib import ExitStack

import ml_dtypes
import numpy as np

B, S, D, H = 2, 2048, 1024, 16
DK = D // H  # 64
N_CORES = 8
HG = H // (N_CORES // B)  # heads per core = 4
NPAIR = HG // 2  # head pairs per core = 2
MT = D // 128  # 8 m-tiles (contraction tiles for projections)
KT = S // 128  # 16 k-tiles
QC = S // 512  # 4 q-chunks

_CACHED = {}


def _build_program():
    import concourse.bass as bass
    import concourse.tile as tile
    from concourse import bacc, mybir

    BF16 = mybir.dt.bfloat16
    F32 = mybir.dt.float32
    AF = mybir.ActivationFunctionType

    nc = bacc.Bacc(trn_type="TRN2", target_bir_lowering=False, debug=False)

    # ---- DRAM I/O (per-core shapes) ----
    xqT = nc.dram_tensor("xqT", [D, S], BF16, kind="ExternalInput").ap()
    xkT = nc.dram_tensor("xkT", [D, S], BF16, kind="ExternalInput").ap()
    xvT = nc.dram_tensor("xvT", [D, S], BF16, kind="ExternalInput").ap()
    wqT = nc.dram_tensor("wqT", [D, 2 * 128], BF16, kind="ExternalInput").ap()
    wkT = nc.dram_tensor("wkT", [D, 2 * 128], BF16, kind="ExternalInput").ap()
    wvT = nc.dram_tensor("wvT", [D, 2 * 128], BF16, kind="ExternalInput").ap()
    woT = nc.dram_tensor("woT", [2 * 128, D], BF16, kind="ExternalInput").ap()
    attnT = nc.dram_tensor("attnT", [HG, S, S], BF16, kind="ExternalOutput").ap()
    outp = nc.dram_tensor("outp", [S, D], F32, kind="ExternalOutput").ap()

    scale = float(1.0 / np.sqrt(np.float32(DK)))

    with tile.TileContext(nc) as tc, ExitStack() as ctx:
        # preload the ACT table set containing BOTH Exp and Ln (and Copy) so
        # the auto-inserted table loads never ping-pong between sets.
        # act_func_set_id 6 = natural_log_exp_and_others (act_info.json).
        nc.scalar.add_instruction(
            mybir.InstLoadActFuncSet(
                name=nc.get_next_instruction_name(),
                ins=[],
                outs=[],
                act_func_set_id=6,
            )
        )

        # ---------- persistent pools ----------
        qk_pool = ctx.enter_context(tc.tile_pool(name="qk", bufs=4))
        vp_pool = ctx.enter_context(tc.tile_pool(name="vp", bufs=HG * KT))
        wo_pool = ctx.enter_context(tc.tile_pool(name="wo", bufs=2))
        ctx_pool = ctx.enter_context(tc.tile_pool(name="ctxT", bufs=2))
        rz_pool = ctx.enter_context(tc.tile_pool(name="rz", bufs=3))
        sm_pool = ctx.enter_context(tc.tile_pool(name="sm", bufs=4))
        stage_pool = ctx.enter_context(tc.tile_pool(name="stage", bufs=3))
        osb_pool = ctx.enter_context(tc.tile_pool(name="osb", bufs=2))

        ps_small = ctx.enter_context(
            tc.tile_pool(name="ps_small", bufs=2, space="PSUM")
        )

        # persistent SBUF tensors
        QT2 = [qk_pool.tile([128, S], BF16, tag="qk", name=f"QT2_{p}") for p in range(NPAIR)]
        KT2 = [qk_pool.tile([128, S], BF16, tag="qk", name=f"KT2_{p}") for p in range(NPAIR)]
        Vp = [
            [vp_pool.tile([128, DK + 1], BF16, tag="vp", name=f"Vp_{h}_{kt}") for kt in range(KT)]
            for h in range(HG)
        ]
        woT_sb = [wo_pool.tile([128, D], BF16, tag="wo", name=f"woT_sb_{i}") for i in range(2)]
        ctxT2 = [ctx_pool.tile([128, S], BF16, tag="ctxT", name=f"ctxT2_{p}") for p in range(NPAIR)]

        for i in range(2):
            nc.sync.dma_start(woT_sb[i][:], woT[128 * i : 128 * (i + 1), :])

        def load_w_tiles(wT, nm, pool):
            tiles = []
            for mt in range(MT):
                t = pool.tile([128, 256], BF16, tag="wt", name=f"wt_{nm}_{mt}")
                nc.sync.dma_start(t[:], wT[128 * mt : 128 * (mt + 1), :])
                tiles.append(t)
            return tiles

        def load_x_tiles(xT, nm, pool):
            tiles = []
            for mt in range(MT):
                t = pool.tile([128, S], BF16, tag="xt", name=f"xt_{nm}_{mt}")
                nc.sync.dma_start(t[:], xT[128 * mt : 128 * (mt + 1), :])
                tiles.append(t)
            return tiles

        # ---------- phase 0: Q^T / K^T projections ----------
        with ExitStack() as p0:
            xt_pool = p0.enter_context(tc.tile_pool(name="xt", bufs=16))
            wt_pool = p0.enter_context(tc.tile_pool(name="wt", bufs=16))

            # weights first (small, needed by the first matmuls), then the
            # big input loads so DMA runs ahead
            wtq = load_w_tiles(wqT, "q", wt_pool)
            wtk = load_w_tiles(wkT, "k", wt_pool)
            xtq = load_x_tiles(xqT, "q", xt_pool)
            xtk = load_x_tiles(xkT, "k", xt_pool)

            # --- Q^T and K^T: out[do, s] = sum_m wT[m, do] * xT[m, s] ---
            for xt, wt, DEST in ((xtq, wtq, QT2), (xtk, wtk, KT2)):
                for p in range(NPAIR):
                    for c in range(QC):
                        ps = ps_small.tile([128, 512], F32, tag="ps_small")
                        for mt in range(MT):
                            nc.tensor.matmul(
                                ps[:],
                                wt[mt][:, 128 * p : 128 * (p + 1)],
                                xt[mt][:, 512 * c : 512 * (c + 1)],
                                start=(mt == 0),
                                stop=(mt == MT - 1),
                            )
                        nc.vector.tensor_copy(
                            DEST[p][:, 512 * c : 512 * (c + 1)], ps[:]
                        )

        def v_block(st):
            # one s-tile of the V projection: out[s, dv] = sum_m xvT * wvT
            ps = ps_small.tile([128, 256], F32, tag="ps_small", name=f"vps_{st}")
            for mt in range(MT):
                nc.tensor.matmul(
                    ps[:],
                    xtv[mt][:, 128 * st : 128 * (st + 1)],
                    wtv[mt][:],
                    start=(mt == 0),
                    stop=(mt == MT - 1),
                )
            for h in range(HG):
                nc.vector.tensor_copy(
                    Vp[h][st][:, 0:DK], ps[:, DK * h : DK * (h + 1)]
                )
                nc.vector.memset(Vp[h][st][:, DK : DK + 1], 1.0)

        # ---------- attention ----------
        with ExitStack() as pa:
            pt_pool_a = pa.enter_context(tc.tile_pool(name="pt_a", bufs=40))
            ps_s = pa.enter_context(tc.tile_pool(name="ps_s", bufs=3, space="PSUM"))
            pt_pools = [pt_pool_a]
            pv0 = ExitStack()
            xv_pool = pv0.enter_context(tc.tile_pool(name="xv", bufs=8))
            wv_pool = pv0.enter_context(tc.tile_pool(name="wv", bufs=8))
            wtv = load_w_tiles(wvT, "v", wv_pool)
            xtv = load_x_tiles(xvT, "v", xv_pool)

            # PT[h][kt][qh]: exp(S^T) bf16 half tiles [128 k, 1024 q]
            PT = [[[None] * 2 for _ in range(KT)] for _ in range(HG)]
            rzrep = {}  # (h, qh) -> [128, 1024] bf16 recipZ replicated

            def pv_job_a(p, hh, c):
                """PV accumulation for chunk c of head 2p+hh, plus Z ->
                recipZ on ACT.  Returns state for pv_job_b."""
                h = 2 * p + hh
                qh, sub = divmod(c, 2)
                pv = ps_small.tile(
                    [DK + 1, 512], F32, tag="ps_small", name=f"pv_{h}_{c}"
                )
                for kt in range(KT):
                    nc.tensor.matmul(
                        pv[:],
                        Vp[h][kt][:],
                        PT[h][kt][qh][:, 512 * sub : 512 * (sub + 1)],
                        start=(kt == 0),
                        stop=(kt == KT - 1),
                    )
                lnz = sm_pool.tile([1, 512], F32, tag="sm", name=f"lnz_{h}_{c}")
                nc.scalar.activation(lnz[:], pv[DK : DK + 1, :], AF.Ln)
                rz = sm_pool.tile([1, 512], F32, tag="sm", name=f"rz_{h}_{c}")
                nc.scalar.activation(rz[:], lnz[:], AF.Exp, scale=-1.0)
                return (p, hh, c, pv, rz)

            def pv_job_b(state):
                """Broadcast recipZ across partitions, normalize ctx^T,
                build bf16 recipZ replica for the attention staging."""
                p, hh, c, pv, rz = state
                h = 2 * p + hh
                qh, sub = divmod(c, 2)
                zrf = sm_pool.tile([128, 512], F32, tag="zrf", name=f"zrf_{h}_{c}")
                nc.gpsimd.partition_broadcast(zrf[:], rz[:])
                nc.vector.tensor_mul(
                    ctxT2[p][64 * hh : 64 * (hh + 1), 512 * c : 512 * (c + 1)],
                    pv[0:DK, :],
                    zrf[0:DK, :],
                )
                key = (h, qh)
                if key not in rzrep:
                    rzrep[key] = rz_pool.tile(
                        [128, 1024], BF16, tag="rz", name=f"rzrep_{h}_{qh}"
                    )
                nc.vector.tensor_copy(
                    rzrep[key][:, 512 * sub : 512 * (sub + 1)], zrf[:]
                )

            def attn_stage(h, qh):
                """Normalize P~^T and DMA the staged transposed attention.
                Alternate DVE / GpSimd so the two elementwise engines split
                the multiply work."""
                for kt in range(KT):
                    at = stage_pool.tile(
                        [128, 1024], BF16, tag="stage", name=f"at_{h}_{kt}_{qh}"
                    )
                    nc.vector.tensor_mul(at[:], PT[h][kt][qh][:], rzrep[(h, qh)][:])
                    nc.sync.dma_start(
                        attnT[
                            h,
                            128 * kt : 128 * (kt + 1),
                            1024 * qh : 1024 * (qh + 1),
                        ],
                        at[:],
                    )

            def scores_phase(p, qh, jobs, fillers=()):
                """S^T + exp for both heads of pair p over q-half qh, with
                the previous q-half's PV/normalize work interleaved.

                jobs: list of (p_prev, hh, c) chunk descriptors.
                fillers: callables emitted (spread out) into the phase."""
                pending_b = None
                points = {1: 0, 4: 1, 7: 2, 10: 3, 13: 4}
                fillers = list(fillers)
                nfill = len(fillers)
                for kt in range(KT):
                    if nfill:
                        lo = nfill * kt // KT
                        hi = nfill * (kt + 1) // KT
                        for fi in range(lo, hi):
                            fillers[fi]()
                    for hh in range(2):
                        h = 2 * p + hh
                        pool = pt_pools[(2 * kt + hh) % len(pt_pools)]
                        ptile = pool.tile(
                            [128, 1024], BF16, tag="pt", name=f"PT_{h}_{kt}_{qh}"
                        )
                        PT[h][kt][qh] = ptile
                        ps = ps_s.tile(
                            [128, 1024], F32, tag="ps_s", name=f"st_{h}_{kt}_{qh}"
                        )
                        for sub in range(2):
                            q0 = 1024 * qh + 512 * sub
                            nc.tensor.matmul(
                                ps[:, 512 * sub : 512 * (sub + 1)],
                                KT2[p][
                                    64 * hh : 64 * (hh + 1),
                                    128 * kt : 128 * (kt + 1),
                                ],
                                QT2[p][64 * hh : 64 * (hh + 1), q0 : q0 + 512],
                                start=True,
                                stop=True,
                            )
                        nc.scalar.activation(ptile[:], ps[:], AF.Exp, scale=scale)
                    if kt in points:
                        j = points[kt]
                        if pending_b is not None:
                            pv_job_b(pending_b)
                            jp, jhh, jc = jobs[j - 1]
                            if jc % 2 == 1:  # second chunk of this head done
                                attn_stage(2 * jp + jhh, jc // 2)
                            pending_b = None
                        if j < len(jobs):
                            pending_b = pv_job_a(*jobs[j])
                # drain leftovers (the j=4 point only handles job 3's b-half)
                if pending_b is not None:
                    pv_job_b(pending_b)
                    jp, jhh, jc = jobs[-1]
                    if jc % 2 == 1:
                        attn_stage(2 * jp + jhh, jc // 2)

            def jobs_for(p, qh):
                return [
                    (p, 0, 2 * qh + 0),
                    (p, 0, 2 * qh + 1),
                    (p, 1, 2 * qh + 0),
                    (p, 1, 2 * qh + 1),
                ]

            def outproj(qts):
                for qt in qts:
                    for mc in range(2):
                        ps = ps_small.tile(
                            [128, 512], F32, tag="ps_small", name=f"ops_{qt}_{mc}"
                        )
                        for p in range(NPAIR):
                            nc.tensor.matmul(
                                ps[:],
                                ctxT2[p][:, 128 * qt : 128 * (qt + 1)],
                                woT_sb[p][:, 512 * mc : 512 * (mc + 1)],
                                start=(p == 0),
                                stop=(p == NPAIR - 1),
                            )
                        osb = osb_pool.tile(
                            [128, 512], F32, tag="osb", name=f"osb_{qt}_{mc}"
                        )
                        nc.scalar.copy(osb[:], ps[:])
                        nc.sync.dma_start(
                            outp[
                                128 * qt : 128 * (qt + 1),
                                512 * mc : 512 * (mc + 1),
                            ],
                            osb[:],
                        )

            import functools

            scores_phase(
                0, 0, [], fillers=[functools.partial(v_block, st) for st in range(KT)]
            )
            pv0.close()  # xv/wv freed
            scores_phase(0, 1, jobs_for(0, 0))
            scores_phase(1, 0, jobs_for(0, 1))
            scores_phase(1, 1, jobs_for(1, 0))
            # drain: PV for the last q-half, interleaved with the output
            # projection for the q-tiles whose ctx columns are already final
            # (chunks 0-1 = q-tiles 0-7 completed during the last phase).
            jobs = jobs_for(1, 1)
            st0 = pv_job_a(*jobs[0])
            st1 = pv_job_a(*jobs[1])
            pv_job_b(st0)
            outproj(range(0, 3))
            pv_job_b(st1)
            st2 = pv_job_a(*jobs[2])
            attn_stage(2 * jobs[1][0] + jobs[1][1], 1)
            outproj(range(3, 6))
            st3 = pv_job_a(*jobs[3])
            pv_job_b(st2)
            outproj(range(6, 8))
            pv_job_b(st3)
            attn_stage(2 * jobs[3][0] + jobs[3][1], 1)
            outproj(range(8, KT))

    nc.compile()
    return nc


def _get_program():
    if "nc" not in _CACHED:
        _CACHED["nc"] = _build_program()
    return _CACHED["nc"]


def _reference_numpy(query, key, value, mask, w_q, w_k, w_v, w_o, b_o):
    """Fallback for inputs the fast path does not handle (masked rows)."""
    scale = np.sqrt(np.float32(DK))
    q = np.asarray(query, np.float32)
    k = np.asarray(key, np.float32)
    v = np.asarray(value, np.float32)
    Q = (q @ np.asarray(w_q, np.float32).T).reshape(B, S, H, DK).transpose(0, 2, 1, 3)
    K = (k @ np.asarray(w_k, np.float32).T).reshape(B, S, H, DK).transpose(0, 2, 1, 3)
    V = (v @ np.asarray(w_v, np.float32).T).reshape(B, S, H, DK).transpose(0, 2, 1, 3)
    scores = np.einsum("bhqd,bhkd->bhqk", Q, K) / scale
    m = np.asarray(mask)[:, None, None, :]
    scores = np.where(m == 0, np.float32(-1e9), scores)
    scores -= scores.max(-1, keepdims=True)
    e = np.exp(scores)
    attn = (e / e.sum(-1, keepdims=True)).astype(np.float32)
    ctx = np.einsum("bhqk,bhkd->bhqd", attn, V)
    ctx = ctx.transpose(0, 2, 1, 3).reshape(B, S, D)
    out = (ctx @ np.asarray(w_o, np.float32).T + np.asarray(b_o, np.float32)).astype(
        np.float32
    )
    return out, attn


def kernel(query, key, value, mask, w_q, w_k, w_v, w_o, b_o):
    from concourse.bass_utils import run_bass_kernel_spmd

    query = np.asarray(query, np.float32)
    key = np.asarray(key, np.float32)
    value = np.asarray(value, np.float32)
    mask_np = np.asarray(mask)
    if not np.all(mask_np == 1):
        return _reference_numpy(query, key, value, mask_np, w_q, w_k, w_v, w_o, b_o)
    w_q = np.asarray(w_q, np.float32)
    w_k = np.asarray(w_k, np.float32)
    w_v = np.asarray(w_v, np.float32)
    w_o = np.asarray(w_o, np.float32)
    b_o = np.asarray(b_o, np.float32)

    nc = _get_program()

    bf = ml_dtypes.bfloat16
    in_maps = []
    for c in range(N_CORES):
        b, g = divmod(c, N_CORES // B)
        hs = slice(256 * g, 256 * (g + 1))
        in_maps.append(
            {
                "xqT": np.ascontiguousarray(query[b].T).astype(bf),
                "xkT": np.ascontiguousarray(key[b].T).astype(bf),
                "xvT": np.ascontiguousarray(value[b].T).astype(bf),
                "wqT": np.ascontiguousarray(w_q[hs, :].T).astype(bf),
                "wkT": np.ascontiguousarray(w_k[hs, :].T).astype(bf),
                "wvT": np.ascontiguousarray(w_v[hs, :].T).astype(bf),
                "woT": np.ascontiguousarray(w_o[:, hs].T).astype(bf),
            }
        )

    trace = bool(int(os.environ.get("TRN_TRACE", "0")))
    res = run_bass_kernel_spmd(nc, in_maps, list(range(N_CORES)), trace=trace)
    if trace and res.exec_time_ns is not None:
        print(f"HW exec time: {res.exec_time_ns} ns")
        _CACHED["exec_time_ns"] = res.exec_time_ns
        _CACHED["trace"] = res.instructions_and_trace

    out = np.empty((B, S, D), np.float32)
    attn = np.empty((B, H, S, S), np.float32)
    partials = np.zeros((B, S, D), np.float32)
    for c in range(N_CORES):
        b, g = divmod(c, N_CORES // B)
        r = res.results[c]
        partials[b] += r["outp"]
        a = r["attnT"]  # [HG, S(k), S(q)] bf16
        for j in range(HG):
            attn[b, HG * g + j] = a[j].T.astype(np.float32)
    for b in range(B):
        out[b] = partials[b] + b_o
    return out, attn


# revision 16
# speedup vs baseline: 1.0818x; 1.0289x over previous
"""MultiHeadAttention forward on 8 trn2 NeuronCores (Bass/Tile).

Problem: B=2, S=2048, D=1024, H=16, DK=64.  reference returns (out, attn):
  Q = q @ Wq.T ; K = k @ Wk.T ; V = v @ Wv.T   (per-head split)
  S = Q K^T / sqrt(DK) ; A = softmax(S) ; ctx = A V ; out = ctx @ Wo.T + b_o

Sharding: data-parallel over B (2) x tensor-parallel over heads (4 groups of
4 heads) = 8 cores.  Each core computes its 4 heads' attention entirely in the
transposed orientation (S^T with k on partitions), which gives softmax row
sums for free via a ones-column appended to V during the PV matmul, and needs
only ONE exp pass on the scalar engine.  The normalized attention is staged
TRANSPOSED ([head, k, q]) in bf16; the host transposes/upcasts during the
unshard step.  Output projection partial products (one per head group) are
summed on the host during the gather.

All GEMMs run in bf16 with fp32 PSUM accumulation.  Scores matmuls pack the
two heads of a pair into disjoint PE row groups (K=64 each) so they run
concurrently.  PV/attention work for one q-half is interleaved into the next
q-half's scores stream to keep the PE dense.
"""

import os
import sys

sys.path.insert(0, "/opt/trn_rl_repo")

from contextlib import ExitStack

import ml_dtypes
import numpy as np

B, S, D, H = 2, 2048, 1024, 16
DK = D // H  # 64
N_CORES = 8
HG = H // (N_CORES // B)  # heads per core = 4
NPAIR = HG // 2  # head pairs per core = 2
MT = D // 128  # 8 m-tiles (contraction tiles for projections)
KT = S // 128  # 16 k-tiles
QC = S // 512  # 4 q-chunks

_CACHED = {}


def _build_program():
    import concourse.bass as bass
    import concourse.tile as tile
    from concourse import bacc, mybir

    BF16 = mybir.dt.bfloat16
    F32 = mybir.dt.float32
    AF = mybir.ActivationFunctionType

    nc = bacc.Bacc(trn_type="TRN2", target_bir_lowering=False, debug=False)

    # ---- DRAM I/O (per-core shapes) ----
    xqT = nc.dram_tensor("xqT", [D, S], BF16, kind="ExternalInput").ap()
    xkT = nc.dram_tensor("xkT", [D, S], BF16, kind="ExternalInput").ap()
    xvT = nc.dram_tensor("xvT", [D, S], BF16, kind="ExternalInput").ap()
    wqT = nc.dram_tensor("wqT", [D, 2 * 128], BF16, kind="ExternalInput").ap()
    wkT = nc.dram_tensor("wkT", [D, 2 * 128], BF16, kind="ExternalInput").ap()
    wvT = nc.dram_tensor("wvT", [D, 2 * 128], BF16, kind="ExternalInput").ap()
    woT = nc.dram_tensor("woT", [2 * 128, D], BF16, kind="ExternalInput").ap()
    attnT = nc.dram_tensor("attnT", [HG, S, S], BF16, kind="ExternalOutput").ap()
    outp = nc.dram_tensor("outp", [S, D], F32, kind="ExternalOutput").ap()

    scale = float(1.0 / np.sqrt(np.float32(DK)))

    with tile.TileContext(nc) as tc, ExitStack() as ctx:
        # preload the ACT table set containing BOTH Exp and Ln (and Copy) so
        # the auto-inserted table loads never ping-pong between sets.
        # act_func_set_id 6 = natural_log_exp_and_others (act_info.json).
        nc.scalar.add_instruction(
            mybir.InstLoadActFuncSet(
                name=nc.get_next_instruction_name(),
                ins=[],
                outs=[],
                act_func_set_id=6,
            )
        )

        # ---------- persistent pools ----------
        qk_pool = ctx.enter_context(tc.tile_pool(name="qk", bufs=4))
        vp_pool = ctx.enter_context(tc.tile_pool(name="vp", bufs=HG))
        wo_pool = ctx.enter_context(tc.tile_pool(name="wo", bufs=2))
        ctx_pool = ctx.enter_context(tc.tile_pool(name="ctxT", bufs=2))
        rz_pool = ctx.enter_context(tc.tile_pool(name="rz", bufs=3))
        sm_pool = ctx.enter_context(tc.tile_pool(name="sm", bufs=3))
        stage_pool = ctx.enter_context(tc.tile_pool(name="stage", bufs=3))
        osb_pool = ctx.enter_context(tc.tile_pool(name="osb", bufs=2))

        ps_small = ctx.enter_context(
            tc.tile_pool(name="ps_small", bufs=2, space="PSUM")
        )

        # persistent SBUF tensors
        QT2 = [qk_pool.tile([128, S], BF16, tag="qk", name=f"QT2_{p}") for p in range(NPAIR)]
        KT2 = [qk_pool.tile([128, S], BF16, tag="qk", name=f"KT2_{p}") for p in range(NPAIR)]
        VpT = [
            vp_pool.tile([128, KT * (DK + 1)], BF16, tag="vp", name=f"Vp_{h}")
            for h in range(HG)
        ]
        Vp = [
            [VpT[h][:, (DK + 1) * kt : (DK + 1) * (kt + 1)] for kt in range(KT)]
            for h in range(HG)
        ]
        woT_sb = [wo_pool.tile([128, D], BF16, tag="wo", name=f"woT_sb_{i}") for i in range(2)]
        ctxT2 = [ctx_pool.tile([128, S], BF16, tag="ctxT", name=f"ctxT2_{p}") for p in range(NPAIR)]

        for i in range(2):
            nc.sync.dma_start(woT_sb[i][:], woT[128 * i : 128 * (i + 1), :])

        def load_w_tiles(wT, nm, pool):
            tiles = []
            for mt in range(MT):
                t = pool.tile([128, 256], BF16, tag="wt", name=f"wt_{nm}_{mt}")
                nc.sync.dma_start(t[:], wT[128 * mt : 128 * (mt + 1), :])
                tiles.append(t)
            return tiles

        def load_x_tiles(xT, nm, pool):
            tiles = []
            for mt in range(MT):
                t = pool.tile([128, S], BF16, tag="xt", name=f"xt_{nm}_{mt}")
                nc.sync.dma_start(t[:], xT[128 * mt : 128 * (mt + 1), :])
                tiles.append(t)
            return tiles

        # ---------- phase 0: Q^T / K^T projections ----------
        with ExitStack() as p0:
            xt_pool = p0.enter_context(tc.tile_pool(name="xt", bufs=16))
            wt_pool = p0.enter_context(tc.tile_pool(name="wt", bufs=16))

            # weights first (small, needed by the first matmuls), then the
            # big input loads so DMA runs ahead
            wtq = load_w_tiles(wqT, "q", wt_pool)
            wtk = load_w_tiles(wkT, "k", wt_pool)
            xtq = load_x_tiles(xqT, "q", xt_pool)
            xtk = load_x_tiles(xkT, "k", xt_pool)

            # --- Q^T and K^T: out[do, s] = sum_m wT[m, do] * xT[m, s] ---
            for xt, wt, DEST in ((xtq, wtq, QT2), (xtk, wtk, KT2)):
                for p in range(NPAIR):
                    for c in range(QC):
                        ps = ps_small.tile([128, 512], F32, tag="ps_small")
                        for mt in range(MT):
                            nc.tensor.matmul(
                                ps[:],
                                wt[mt][:, 128 * p : 128 * (p + 1)],
                                xt[mt][:, 512 * c : 512 * (c + 1)],
                                start=(mt == 0),
                                stop=(mt == MT - 1),
                            )
                        nc.vector.tensor_copy(
                            DEST[p][:, 512 * c : 512 * (c + 1)], ps[:]
                        )

        def v_block(st):
            # one s-tile of the V projection: out[s, dv] = sum_m xvT * wvT
            ps = ps_small.tile([128, 256], F32, tag="ps_small", name=f"vps_{st}")
            for mt in range(MT):
                nc.tensor.matmul(
                    ps[:],
                    xtv[mt][:, 128 * st : 128 * (st + 1)],
                    wtv[mt][:],
                    start=(mt == 0),
                    stop=(mt == MT - 1),
                )
            for h in range(HG):
                nc.vector.tensor_copy(
                    Vp[h][st][:, 0:DK], ps[:, DK * h : DK * (h + 1)]
                )
                nc.vector.memset(Vp[h][st][:, DK : DK + 1], 1.0)

        # ---------- attention ----------
        with ExitStack() as pa:
            pt_pool_a = pa.enter_context(tc.tile_pool(name="pt_a", bufs=46))
            ps_s = pa.enter_context(tc.tile_pool(name="ps_s", bufs=3, space="PSUM"))
            pt_pools = [pt_pool_a]
            pv0 = ExitStack()
            xv_pool = pv0.enter_context(tc.tile_pool(name="xv", bufs=8))
            wv_pool = pv0.enter_context(tc.tile_pool(name="wv", bufs=8))
            wtv = load_w_tiles(wvT, "v", wv_pool)
            xtv = load_x_tiles(xvT, "v", xv_pool)

            # PT[h][kt][qh]: exp(S^T) bf16 half tiles [128 k, 1024 q]
            PT = [[[None] * 2 for _ in range(KT)] for _ in range(HG)]
            rzrep = {}  # (h, qh) -> [128, 1024] bf16 recipZ replicated

            def pv_job_a(p, hh, c):
                """PV accumulation for chunk c of head 2p+hh, plus Z ->
                recipZ on ACT.  Returns state for pv_job_b."""
                h = 2 * p + hh
                qh, sub = divmod(c, 2)
                pv = ps_small.tile(
                    [DK + 1, 512], F32, tag="ps_small", name=f"pv_{h}_{c}"
                )
                for kt in range(KT):
                    nc.tensor.matmul(
                        pv[:],
                        Vp[h][kt][:],
                        PT[h][kt][qh][:, 512 * sub : 512 * (sub + 1)],
                        start=(kt == 0),
                        stop=(kt == KT - 1),
                    )
                lnz = sm_pool.tile([1, 512], F32, tag="sm", name=f"lnz_{h}_{c}")
                nc.scalar.activation(lnz[:], pv[DK : DK + 1, :], AF.Ln)
                rz = sm_pool.tile([1, 512], F32, tag="sm", name=f"rz_{h}_{c}")
                nc.scalar.activation(rz[:], lnz[:], AF.Exp, scale=-1.0)
                return (p, hh, c, pv, rz)

            def pv_job_b(state):
                """Broadcast recipZ across partitions, normalize ctx^T,
                build bf16 recipZ replica for the attention staging."""
                p, hh, c, pv, rz = state
                h = 2 * p + hh
                qh, sub = divmod(c, 2)
                zrf = sm_pool.tile([128, 512], F32, tag="zrf", name=f"zrf_{h}_{c}")
                nc.gpsimd.partition_broadcast(zrf[:], rz[:])
                nc.vector.tensor_mul(
                    ctxT2[p][64 * hh : 64 * (hh + 1), 512 * c : 512 * (c + 1)],
                    pv[0:DK, :],
                    zrf[0:DK, :],
                )
                key = (h, qh)
                if key not in rzrep:
                    rzrep[key] = rz_pool.tile(
                        [128, 1024], BF16, tag="rz", name=f"rzrep_{h}_{qh}"
                    )
                nc.vector.tensor_copy(
                    rzrep[key][:, 512 * sub : 512 * (sub + 1)], zrf[:]
                )

            def attn_stage(h, qh):
                """Normalize P~^T and DMA the staged transposed attention.
                Alternate DVE / GpSimd so the two elementwise engines split
                the multiply work."""
                for kt in range(KT):
                    at = stage_pool.tile(
                        [128, 1024], BF16, tag="stage", name=f"at_{h}_{kt}_{qh}"
                    )
                    nc.vector.tensor_mul(at[:], PT[h][kt][qh][:], rzrep[(h, qh)][:])
                    nc.sync.dma_start(
                        attnT[
                            h,
                            128 * kt : 128 * (kt + 1),
                            1024 * qh : 1024 * (qh + 1),
                        ],
                        at[:],
                    )

            def scores_phase(p, qh, jobs, fillers=()):
                """S^T + exp for both heads of pair p over q-half qh, with
                the previous q-half's PV/normalize work interleaved.

                jobs: list of (p_prev, hh, c) chunk descriptors.
                fillers: callables emitted (spread out) into the phase."""
                pending_b = None
                points = {1: 0, 4: 1, 7: 2, 10: 3, 13: 4}
                fillers = list(fillers)
                nfill = len(fillers)
                for kt in range(KT):
                    if nfill:
                        lo = nfill * kt // KT
                        hi = nfill * (kt + 1) // KT
                        for fi in range(lo, hi):
                            fillers[fi]()
                    for hh in range(2):
                        h = 2 * p + hh
                        pool = pt_pools[(2 * kt + hh) % len(pt_pools)]
                        ptile = pool.tile(
                            [128, 1024], BF16, tag="pt", name=f"PT_{h}_{kt}_{qh}"
                        )
                        PT[h][kt][qh] = ptile
                        ps = ps_s.tile(
                            [128, 1024], F32, tag="ps_s", name=f"st_{h}_{kt}_{qh}"
                        )
                        for sub in range(2):
                            q0 = 1024 * qh + 512 * sub
                            nc.tensor.matmul(
                                ps[:, 512 * sub : 512 * (sub + 1)],
                                KT2[p][
                                    64 * hh : 64 * (hh + 1),
                                    128 * kt : 128 * (kt + 1),
                                ],
                                QT2[p][64 * hh : 64 * (hh + 1), q0 : q0 + 512],
                                start=True,
                                stop=True,
                            )
                        nc.scalar.activation(ptile[:], ps[:], AF.Exp, scale=scale)
                    if kt in points:
                        j = points[kt]
                        if pending_b is not None:
                            pv_job_b(pending_b)
                            jp, jhh, jc = jobs[j - 1]
                            if jc % 2 == 1:  # second chunk of this head done
                                attn_stage(2 * jp + jhh, jc // 2)
                            pending_b = None
                        if j < len(jobs):
                            pending_b = pv_job_a(*jobs[j])
                # drain leftovers (the j=4 point only handles job 3's b-half)
                if pending_b is not None:
                    pv_job_b(pending_b)
                    jp, jhh, jc = jobs[-1]
                    if jc % 2 == 1:
                        attn_stage(2 * jp + jhh, jc // 2)

            def jobs_for(p, qh):
                return [
                    (p, 0, 2 * qh + 0),
                    (p, 0, 2 * qh + 1),
                    (p, 1, 2 * qh + 0),
                    (p, 1, 2 * qh + 1),
                ]

            def outproj(qts):
                for qt in qts:
                    for mc in range(2):
                        ps = ps_small.tile(
                            [128, 512], F32, tag="ps_small", name=f"ops_{qt}_{mc}"
                        )
                        for p in range(NPAIR):
                            nc.tensor.matmul(
                                ps[:],
                                ctxT2[p][:, 128 * qt : 128 * (qt + 1)],
                                woT_sb[p][:, 512 * mc : 512 * (mc + 1)],
                                start=(p == 0),
                                stop=(p == NPAIR - 1),
                            )
                        osb = osb_pool.tile(
                            [128, 512], F32, tag="osb", name=f"osb_{qt}_{mc}"
                        )
                        nc.scalar.copy(osb[:], ps[:])
                        nc.sync.dma_start(
                            outp[
                                128 * qt : 128 * (qt + 1),
                                512 * mc : 512 * (mc + 1),
                            ],
                            osb[:],
                        )

            import functools

            scores_phase(
                0, 0, [], fillers=[functools.partial(v_block, st) for st in range(KT)]
            )
            pv0.close()  # xv/wv freed
            scores_phase(0, 1, jobs_for(0, 0))
            scores_phase(1, 0, jobs_for(0, 1))
            scores_phase(1, 1, jobs_for(1, 0))
            # drain: PV for the last q-half, interleaved with the output
            # projection for the q-tiles whose ctx columns are already final
            # (chunks 0-1 = q-tiles 0-7 completed during the last phase).
            jobs = jobs_for(1, 1)
            st0 = pv_job_a(*jobs[0])
            st1 = pv_job_a(*jobs[1])
            pv_job_b(st0)
            outproj(range(0, 3))
            pv_job_b(st1)
            st2 = pv_job_a(*jobs[2])
            attn_stage(2 * jobs[1][0] + jobs[1][1], 1)
            outproj(range(3, 6))
            st3 = pv_job_a(*jobs[3])
            pv_job_b(st2)
            outproj(range(6, 8))
            pv_job_b(st3)
            attn_stage(2 * jobs[3][0] + jobs[3][1], 1)
            outproj(range(8, KT))

    nc.compile()
    return nc


def _get_program():
    if "nc" not in _CACHED:
        _CACHED["nc"] = _build_program()
    return _CACHED["nc"]


def _reference_numpy(query, key, value, mask, w_q, w_k, w_v, w_o, b_o):
    """Fallback for inputs the fast path does not handle (masked rows)."""
    scale = np.sqrt(np.float32(DK))
    q = np.asarray(query, np.float32)
    k = np.asarray(key, np.float32)
    v = np.asarray(value, np.float32)
    Q = (q @ np.asarray(w_q, np.float32).T).reshape(B, S, H, DK).transpose(0, 2, 1, 3)
    K = (k @ np.asarray(w_k, np.float32).T).reshape(B, S, H, DK).transpose(0, 2, 1, 3)
    V = (v @ np.asarray(w_v, np.float32).T).reshape(B, S, H, DK).transpose(0, 2, 1, 3)
    scores = np.einsum("bhqd,bhkd->bhqk", Q, K) / scale
    m = np.asarray(mask)[:, None, None, :]
    scores = np.where(m == 0, np.float32(-1e9), scores)
    scores -= scores.max(-1, keepdims=True)
    e = np.exp(scores)
    attn = (e / e.sum(-1, keepdims=True)).astype(np.float32)
    ctx = np.einsum("bhqk,bhkd->bhqd", attn, V)
    ctx = ctx.transpose(0, 2, 1, 3).reshape(B, S, D)
    out = (ctx @ np.asarray(w_o, np.float32).T + np.asarray(b_o, np.float32)).astype(
        np.float32
    )
    return out, attn


def kernel(query, key, value, mask, w_q, w_k, w_v, w_o, b_o):
    from concourse.bass_utils import run_bass_kernel_spmd

    query = np.asarray(query, np.float32)
    key = np.asarray(key, np.float32)
    value = np.asarray(value, np.float32)
    mask_np = np.asarray(mask)
    if not np.all(mask_np == 1):
        return _reference_numpy(query, key, value, mask_np, w_q, w_k, w_v, w_o, b_o)
    w_q = np.asarray(w_q, np.float32)
    w_k = np.asarray(w_k, np.float32)
    w_v = np.asarray(w_v, np.float32)
    w_o = np.asarray(w_o, np.float32)
    b_o = np.asarray(b_o, np.float32)

    nc = _get_program()

    bf = ml_dtypes.bfloat16
    in_maps = []
    for c in range(N_CORES):
        b, g = divmod(c, N_CORES // B)
        hs = slice(256 * g, 256 * (g + 1))
        in_maps.append(
            {
                "xqT": np.ascontiguousarray(query[b].T).astype(bf),
                "xkT": np.ascontiguousarray(key[b].T).astype(bf),
                "xvT": np.ascontiguousarray(value[b].T).astype(bf),
                "wqT": np.ascontiguousarray(w_q[hs, :].T).astype(bf),
                "wkT": np.ascontiguousarray(w_k[hs, :].T).astype(bf),
                "wvT": np.ascontiguousarray(w_v[hs, :].T).astype(bf),
                "woT": np.ascontiguousarray(w_o[:, hs].T).astype(bf),
            }
        )

    trace = bool(int(os.environ.get("TRN_TRACE", "0")))
    res = run_bass_kernel_spmd(nc, in_maps, list(range(N_CORES)), trace=trace)
    if trace and res.exec_time_ns is not None:
        print(f"HW exec time: {res.exec_time_ns} ns")
        _CACHED["exec_time_ns"] = res.exec_time_ns
        _CACHED["trace"] = res.instructions_and_trace

    out = np.empty((B, S, D), np.float32)
    attn = np.empty((B, H, S, S), np.float32)
    partials = np.zeros((B, S, D), np.float32)
    for c in range(N_CORES):
        b, g = divmod(c, N_CORES // B)
        r = res.results[c]
        partials[b] += r["outp"]
        a = r["attnT"]  # [HG, S(k), S(q)] bf16
        for j in range(HG):
            attn[b, HG * g + j] = a[j].T.astype(np.float32)
    for b in range(B):
        out[b] = partials[b] + b_o
    return out, attn


# revision 17
# speedup vs baseline: 1.0859x; 1.0039x over previous
"""MultiHeadAttention forward on 8 trn2 NeuronCores (Bass/Tile).

Problem: B=2, S=2048, D=1024, H=16, DK=64.  reference returns (out, attn):
  Q = q @ Wq.T ; K = k @ Wk.T ; V = v @ Wv.T   (per-head split)
  S = Q K^T / sqrt(DK) ; A = softmax(S) ; ctx = A V ; out = ctx @ Wo.T + b_o

Sharding: data-parallel over B (2) x tensor-parallel over heads (4 groups of
4 heads) = 8 cores.  Each core computes its 4 heads' attention entirely in the
transposed orientation (S^T with k on partitions), which gives softmax row
sums for free via a ones-column appended to V during the PV matmul, and needs
only ONE exp pass on the scalar engine.  The normalized attention is staged
TRANSPOSED ([head, k, q]) in bf16; the host transposes/upcasts during the
unshard step.  Output projection partial products (one per head group) are
summed on the host during the gather.

All GEMMs run in bf16 with fp32 PSUM accumulation.  Scores matmuls pack the
two heads of a pair into disjoint PE row groups (K=64 each) so they run
concurrently.  PV/attention work for one q-half is interleaved into the next
q-half's scores stream to keep the PE dense.
"""

import os
import sys

sys.path.insert(0, "/opt/trn_rl_repo")

from contextlib import ExitStack

import ml_dtypes
import numpy as np

B, S, D, H = 2, 2048, 1024, 16
DK = D // H  # 64
N_CORES = 8
HG = H // (N_CORES // B)  # heads per core = 4
NPAIR = HG // 2  # head pairs per core = 2
MT = D // 128  # 8 m-tiles (contraction tiles for projections)
KT = S // 128  # 16 k-tiles
QC = S // 512  # 4 q-chunks

_CACHED = {}


def _build_program():
    import concourse.bass as bass
    import concourse.tile as tile
    from concourse import bacc, mybir

    BF16 = mybir.dt.bfloat16
    F32 = mybir.dt.float32
    AF = mybir.ActivationFunctionType

    nc = bacc.Bacc(trn_type="TRN2", target_bir_lowering=False, debug=False)

    # ---- DRAM I/O (per-core shapes) ----
    xqT = nc.dram_tensor("xqT", [D, S], BF16, kind="ExternalInput").ap()
    xkT = nc.dram_tensor("xkT", [D, S], BF16, kind="ExternalInput").ap()
    xvT = nc.dram_tensor("xvT", [D, S], BF16, kind="ExternalInput").ap()
    wqT = nc.dram_tensor("wqT", [D, 2 * 128], BF16, kind="ExternalInput").ap()
    wkT = nc.dram_tensor("wkT", [D, 2 * 128], BF16, kind="ExternalInput").ap()
    wvT = nc.dram_tensor("wvT", [D, 2 * 128], BF16, kind="ExternalInput").ap()
    woT = nc.dram_tensor("woT", [2 * 128, D], BF16, kind="ExternalInput").ap()
    attnT = nc.dram_tensor("attnT", [HG, S, S], BF16, kind="ExternalOutput").ap()
    outp = nc.dram_tensor("outp", [S, D], F32, kind="ExternalOutput").ap()

    scale = float(1.0 / np.sqrt(np.float32(DK)))

    with tile.TileContext(nc) as tc, ExitStack() as ctx:
        # preload the ACT table set containing BOTH Exp and Ln (and Copy) so
        # the auto-inserted table loads never ping-pong between sets.
        # act_func_set_id 6 = natural_log_exp_and_others (act_info.json).
        nc.scalar.add_instruction(
            mybir.InstLoadActFuncSet(
                name=nc.get_next_instruction_name(),
                ins=[],
                outs=[],
                act_func_set_id=6,
            )
        )

        # ---------- persistent pools ----------
        qk_pool = ctx.enter_context(tc.tile_pool(name="qk", bufs=4))
        vp_pool = ctx.enter_context(tc.tile_pool(name="vp", bufs=HG))
        wo_pool = ctx.enter_context(tc.tile_pool(name="wo", bufs=2))
        ctx_pool = ctx.enter_context(tc.tile_pool(name="ctxT", bufs=2))
        rz_pool = ctx.enter_context(tc.tile_pool(name="rz", bufs=3))
        sm_pool = ctx.enter_context(tc.tile_pool(name="sm", bufs=3))
        stage_pool = ctx.enter_context(tc.tile_pool(name="stage", bufs=3))
        osb_pool = ctx.enter_context(tc.tile_pool(name="osb", bufs=2))

        ps_small = ctx.enter_context(
            tc.tile_pool(name="ps_small", bufs=2, space="PSUM")
        )

        # persistent SBUF tensors
        QT2 = [qk_pool.tile([128, S], BF16, tag="qk", name=f"QT2_{p}") for p in range(NPAIR)]
        KT2 = [qk_pool.tile([128, S], BF16, tag="qk", name=f"KT2_{p}") for p in range(NPAIR)]
        VpT = [
            vp_pool.tile([128, KT * (DK + 1)], BF16, tag="vp", name=f"Vp_{h}")
            for h in range(HG)
        ]
        Vp = [
            [VpT[h][:, (DK + 1) * kt : (DK + 1) * (kt + 1)] for kt in range(KT)]
            for h in range(HG)
        ]
        woT_sb = [wo_pool.tile([128, D], BF16, tag="wo", name=f"woT_sb_{i}") for i in range(2)]
        ctxT2 = [ctx_pool.tile([128, S], BF16, tag="ctxT", name=f"ctxT2_{p}") for p in range(NPAIR)]

        for i in range(2):
            nc.sync.dma_start(woT_sb[i][:], woT[128 * i : 128 * (i + 1), :])

        def load_w_tiles(wT, nm, pool):
            tiles = []
            for mt in range(MT):
                t = pool.tile([128, 256], BF16, tag="wt", name=f"wt_{nm}_{mt}")
                nc.sync.dma_start(t[:], wT[128 * mt : 128 * (mt + 1), :])
                tiles.append(t)
            return tiles

        def load_x_tiles(xT, nm, pool):
            tiles = []
            for mt in range(MT):
                t = pool.tile([128, S], BF16, tag="xt", name=f"xt_{nm}_{mt}")
                nc.sync.dma_start(t[:], xT[128 * mt : 128 * (mt + 1), :])
                tiles.append(t)
            return tiles

        # ---------- phase 0: Q^T / K^T projections ----------
        with ExitStack() as p0:
            xt_pool = p0.enter_context(tc.tile_pool(name="xt", bufs=16))
            wt_pool = p0.enter_context(tc.tile_pool(name="wt", bufs=16))

            # weights first (small, needed by the first matmuls), then the
            # big input loads so DMA runs ahead
            wtq = load_w_tiles(wqT, "q", wt_pool)
            wtk = load_w_tiles(wkT, "k", wt_pool)
            xtq = load_x_tiles(xqT, "q", xt_pool)
            xtk = load_x_tiles(xkT, "k", xt_pool)

            # --- Q^T and K^T: out[do, s] = sum_m wT[m, do] * xT[m, s] ---
            for xt, wt, DEST in ((xtq, wtq, QT2), (xtk, wtk, KT2)):
                for p in range(NPAIR):
                    for c in range(QC):
                        ps = ps_small.tile([128, 512], F32, tag="ps_small")
                        for mt in range(MT):
                            nc.tensor.matmul(
                                ps[:],
                                wt[mt][:, 128 * p : 128 * (p + 1)],
                                xt[mt][:, 512 * c : 512 * (c + 1)],
                                start=(mt == 0),
                                stop=(mt == MT - 1),
                            )
                        nc.vector.tensor_copy(
                            DEST[p][:, 512 * c : 512 * (c + 1)], ps[:]
                        )

        def v_block(st):
            # one s-tile of the V projection: out[s, dv] = sum_m xvT * wvT
            ps = ps_small.tile([128, 256], F32, tag="ps_small", name=f"vps_{st}")
            for mt in range(MT):
                nc.tensor.matmul(
                    ps[:],
                    xtv[mt][:, 128 * st : 128 * (st + 1)],
                    wtv[mt][:],
                    start=(mt == 0),
                    stop=(mt == MT - 1),
                )
            for h in range(HG):
                nc.vector.tensor_copy(
                    Vp[h][st][:, 0:DK], ps[:, DK * h : DK * (h + 1)]
                )
                nc.vector.memset(Vp[h][st][:, DK : DK + 1], 1.0)

        # ---------- attention ----------
        with ExitStack() as pa:
            pt_pool_a = pa.enter_context(tc.tile_pool(name="pt_a", bufs=50))
            ps_s = pa.enter_context(tc.tile_pool(name="ps_s", bufs=3, space="PSUM"))
            pt_pools = [pt_pool_a]
            pv0 = ExitStack()
            xv_pool = pv0.enter_context(tc.tile_pool(name="xv", bufs=8))
            wv_pool = pv0.enter_context(tc.tile_pool(name="wv", bufs=8))
            wtv = load_w_tiles(wvT, "v", wv_pool)
            xtv = load_x_tiles(xvT, "v", xv_pool)

            # PT[h][kt][qh]: exp(S^T) bf16 half tiles [128 k, 1024 q]
            PT = [[[None] * 2 for _ in range(KT)] for _ in range(HG)]
            rzrep = {}  # (h, qh) -> [128, 1024] bf16 recipZ replicated

            def pv_job_a(p, hh, c):
                """PV accumulation for chunk c of head 2p+hh, plus Z ->
                recipZ on ACT.  Returns state for pv_job_b."""
                h = 2 * p + hh
                qh, sub = divmod(c, 2)
                pv = ps_small.tile(
                    [DK + 1, 512], F32, tag="ps_small", name=f"pv_{h}_{c}"
                )
                for kt in range(KT):
                    nc.tensor.matmul(
                        pv[:],
                        Vp[h][kt][:],
                        PT[h][kt][qh][:, 512 * sub : 512 * (sub + 1)],
                        start=(kt == 0),
                        stop=(kt == KT - 1),
                    )
                lnz = sm_pool.tile([1, 512], F32, tag="sm", name=f"lnz_{h}_{c}")
                nc.scalar.activation(lnz[:], pv[DK : DK + 1, :], AF.Ln)
                rz = sm_pool.tile([1, 512], F32, tag="sm", name=f"rz_{h}_{c}")
                nc.scalar.activation(rz[:], lnz[:], AF.Exp, scale=-1.0)
                return (p, hh, c, pv, rz)

            def pv_job_b(state):
                """Broadcast recipZ across partitions, normalize ctx^T,
                build bf16 recipZ replica for the attention staging."""
                p, hh, c, pv, rz = state
                h = 2 * p + hh
                qh, sub = divmod(c, 2)
                zrf = sm_pool.tile([128, 512], F32, tag="zrf", name=f"zrf_{h}_{c}")
                nc.gpsimd.partition_broadcast(zrf[:], rz[:])
                nc.vector.tensor_mul(
                    ctxT2[p][64 * hh : 64 * (hh + 1), 512 * c : 512 * (c + 1)],
                    pv[0:DK, :],
                    zrf[0:DK, :],
                )
                key = (h, qh)
                if key not in rzrep:
                    rzrep[key] = rz_pool.tile(
                        [128, 1024], BF16, tag="rz", name=f"rzrep_{h}_{qh}"
                    )
                nc.vector.tensor_copy(
                    rzrep[key][:, 512 * sub : 512 * (sub + 1)], zrf[:]
                )

            def attn_stage(h, qh):
                """Normalize P~^T and DMA the staged transposed attention.
                Alternate DVE / GpSimd so the two elementwise engines split
                the multiply work."""
                for kt in range(KT):
                    at = stage_pool.tile(
                        [128, 1024], BF16, tag="stage", name=f"at_{h}_{kt}_{qh}"
                    )
                    nc.vector.tensor_mul(at[:], PT[h][kt][qh][:], rzrep[(h, qh)][:])
                    nc.sync.dma_start(
                        attnT[
                            h,
                            128 * kt : 128 * (kt + 1),
                            1024 * qh : 1024 * (qh + 1),
                        ],
                        at[:],
                    )

            def scores_phase(p, qh, jobs, fillers=()):
                """S^T + exp for both heads of pair p over q-half qh, with
                the previous q-half's PV/normalize work interleaved.

                jobs: list of (p_prev, hh, c) chunk descriptors.
                fillers: callables emitted (spread out) into the phase."""
                pending_b = None
                points = {1: 0, 4: 1, 7: 2, 10: 3, 13: 4}
                fillers = list(fillers)
                nfill = len(fillers)
                for kt in range(KT):
                    if nfill:
                        lo = nfill * kt // KT
                        hi = nfill * (kt + 1) // KT
                        for fi in range(lo, hi):
                            fillers[fi]()
                    for hh in range(2):
                        h = 2 * p + hh
                        pool = pt_pools[(2 * kt + hh) % len(pt_pools)]
                        ptile = pool.tile(
                            [128, 1024], BF16, tag="pt", name=f"PT_{h}_{kt}_{qh}"
                        )
                        PT[h][kt][qh] = ptile
                        ps = ps_s.tile(
                            [128, 1024], F32, tag="ps_s", name=f"st_{h}_{kt}_{qh}"
                        )
                        for sub in range(2):
                            q0 = 1024 * qh + 512 * sub
                            nc.tensor.matmul(
                                ps[:, 512 * sub : 512 * (sub + 1)],
                                KT2[p][
                                    64 * hh : 64 * (hh + 1),
                                    128 * kt : 128 * (kt + 1),
                                ],
                                QT2[p][64 * hh : 64 * (hh + 1), q0 : q0 + 512],
                                start=True,
                                stop=True,
                            )
                        nc.scalar.activation(ptile[:], ps[:], AF.Exp, scale=scale)
                    if kt in points:
                        j = points[kt]
                        if pending_b is not None:
                            pv_job_b(pending_b)
                            jp, jhh, jc = jobs[j - 1]
                            if jc % 2 == 1:  # second chunk of this head done
                                attn_stage(2 * jp + jhh, jc // 2)
                            pending_b = None
                        if j < len(jobs):
                            pending_b = pv_job_a(*jobs[j])
                # drain leftovers (the j=4 point only handles job 3's b-half)
                if pending_b is not None:
                    pv_job_b(pending_b)
                    jp, jhh, jc = jobs[-1]
                    if jc % 2 == 1:
                        attn_stage(2 * jp + jhh, jc // 2)

            def jobs_for(p, qh):
                return [
                    (p, 0, 2 * qh + 0),
                    (p, 0, 2 * qh + 1),
                    (p, 1, 2 * qh + 0),
                    (p, 1, 2 * qh + 1),
                ]

            def outproj(qts):
                for qt in qts:
                    for mc in range(2):
                        ps = ps_small.tile(
                            [128, 512], F32, tag="ps_small", name=f"ops_{qt}_{mc}"
                        )
                        for p in range(NPAIR):
                            nc.tensor.matmul(
                                ps[:],
                                ctxT2[p][:, 128 * qt : 128 * (qt + 1)],
                                woT_sb[p][:, 512 * mc : 512 * (mc + 1)],
                                start=(p == 0),
                                stop=(p == NPAIR - 1),
                            )
                        osb = osb_pool.tile(
                            [128, 512], F32, tag="osb", name=f"osb_{qt}_{mc}"
                        )
                        nc.scalar.copy(osb[:], ps[:])
                        nc.sync.dma_start(
                            outp[
                                128 * qt : 128 * (qt + 1),
                                512 * mc : 512 * (mc + 1),
                            ],
                            osb[:],
                        )

            import functools

            scores_phase(
                0, 0, [], fillers=[functools.partial(v_block, st) for st in range(KT)]
            )
            pv0.close()  # xv/wv freed
            scores_phase(0, 1, jobs_for(0, 0))
            scores_phase(1, 0, jobs_for(0, 1))
            scores_phase(1, 1, jobs_for(1, 0))
            # drain: PV for the last q-half, interleaved with the output
            # projection for the q-tiles whose ctx columns are already final
            # (chunks 0-1 = q-tiles 0-7 completed during the last phase).
            jobs = jobs_for(1, 1)
            st0 = pv_job_a(*jobs[0])
            st1 = pv_job_a(*jobs[1])
            pv_job_b(st0)
            outproj(range(0, 3))
            pv_job_b(st1)
            st2 = pv_job_a(*jobs[2])
            attn_stage(2 * jobs[1][0] + jobs[1][1], 1)
            outproj(range(3, 6))
            st3 = pv_job_a(*jobs[3])
            pv_job_b(st2)
            outproj(range(6, 8))
            pv_job_b(st3)
            attn_stage(2 * jobs[3][0] + jobs[3][1], 1)
            outproj(range(8, KT))

    nc.compile()
    return nc


def _get_program():
    if "nc" not in _CACHED:
        _CACHED["nc"] = _build_program()
    return _CACHED["nc"]


def _reference_numpy(query, key, value, mask, w_q, w_k, w_v, w_o, b_o):
    """Fallback for inputs the fast path does not handle (masked rows)."""
    scale = np.sqrt(np.float32(DK))
    q = np.asarray(query, np.float32)
    k = np.asarray(key, np.float32)
    v = np.asarray(value, np.float32)
    Q = (q @ np.asarray(w_q, np.float32).T).reshape(B, S, H, DK).transpose(0, 2, 1, 3)
    K = (k @ np.asarray(w_k, np.float32).T).reshape(B, S, H, DK).transpose(0, 2, 1, 3)
    V = (v @ np.asarray(w_v, np.float32).T).reshape(B, S, H, DK).transpose(0, 2, 1, 3)
    scores = np.einsum("bhqd,bhkd->bhqk", Q, K) / scale
    m = np.asarray(mask)[:, None, None, :]
    scores = np.where(m == 0, np.float32(-1e9), scores)
    scores -= scores.max(-1, keepdims=True)
    e = np.exp(scores)
    attn = (e / e.sum(-1, keepdims=True)).astype(np.float32)
    ctx = np.einsum("bhqk,bhkd->bhqd", attn, V)
    ctx = ctx.transpose(0, 2, 1, 3).reshape(B, S, D)
    out = (ctx @ np.asarray(w_o, np.float32).T + np.asarray(b_o, np.float32)).astype(
        np.float32
    )
    return out, attn


def kernel(query, key, value, mask, w_q, w_k, w_v, w_o, b_o):
    from concourse.bass_utils import run_bass_kernel_spmd

    query = np.asarray(query, np.float32)
    key = np.asarray(key, np.float32)
    value = np.asarray(value, np.float32)
    mask_np = np.asarray(mask)
    if not np.all(mask_np == 1):
        return _reference_numpy(query, key, value, mask_np, w_q, w_k, w_v, w_o, b_o)
    w_q = np.asarray(w_q, np.float32)
    w_k = np.asarray(w_k, np.float32)
    w_v = np.asarray(w_v, np.float32)
    w_o = np.asarray(w_o, np.float32)
    b_o = np.asarray(b_o, np.float32)

    nc = _get_program()

    bf = ml_dtypes.bfloat16
    in_maps = []
    for c in range(N_CORES):
        b, g = divmod(c, N_CORES // B)
        hs = slice(256 * g, 256 * (g + 1))
        in_maps.append(
            {
                "xqT": np.ascontiguousarray(query[b].T).astype(bf),
                "xkT": np.ascontiguousarray(key[b].T).astype(bf),
                "xvT": np.ascontiguousarray(value[b].T).astype(bf),
                "wqT": np.ascontiguousarray(w_q[hs, :].T).astype(bf),
                "wkT": np.ascontiguousarray(w_k[hs, :].T).astype(bf),
                "wvT": np.ascontiguousarray(w_v[hs, :].T).astype(bf),
                "woT": np.ascontiguousarray(w_o[:, hs].T).astype(bf),
            }
        )

    trace = bool(int(os.environ.get("TRN_TRACE", "0")))
    res = run_bass_kernel_spmd(nc, in_maps, list(range(N_CORES)), trace=trace)
    if trace and res.exec_time_ns is not None:
        print(f"HW exec time: {res.exec_time_ns} ns")
        _CACHED["exec_time_ns"] = res.exec_time_ns
        _CACHED["trace"] = res.instructions_and_trace

    out = np.empty((B, S, D), np.float32)
    attn = np.empty((B, H, S, S), np.float32)
    partials = np.zeros((B, S, D), np.float32)
    for c in range(N_CORES):
        b, g = divmod(c, N_CORES // B)
        r = res.results[c]
        partials[b] += r["outp"]
        a = r["attnT"]  # [HG, S(k), S(q)] bf16
        for j in range(HG):
            attn[b, HG * g + j] = a[j].T.astype(np.float32)
    for b in range(B):
        out[b] = partials[b] + b_o
    return out, attn


# revision 18
# speedup vs baseline: 1.0960x; 1.0093x over previous
"""MultiHeadAttention forward on 8 trn2 NeuronCores (Bass/Tile).

Problem: B=2, S=2048, D=1024, H=16, DK=64.  reference returns (out, attn):
  Q = q @ Wq.T ; K = k @ Wk.T ; V = v @ Wv.T   (per-head split)
  S = Q K^T / sqrt(DK) ; A = softmax(S) ; ctx = A V ; out = ctx @ Wo.T + b_o

Sharding: data-parallel over B (2) x tensor-parallel over heads (4 groups of
4 heads) = 8 cores.  Each core computes its 4 heads' attention entirely in the
transposed orientation (S^T with k on partitions), which gives softmax row
sums for free via a ones-column appended to V during the PV matmul, and needs
only ONE exp pass on the scalar engine.  The normalized attention is staged
TRANSPOSED ([head, k, q]) in bf16; the host transposes/upcasts during the
unshard step.  Output projection partial products (one per head group) are
summed on the host during the gather.

All GEMMs run in bf16 with fp32 PSUM accumulation.  Scores matmuls pack the
two heads of a pair into disjoint PE row groups (K=64 each) so they run
concurrently.  PV/attention work for one q-half is interleaved into the next
q-half's scores stream to keep the PE dense.
"""

import os
import sys

sys.path.insert(0, "/opt/trn_rl_repo")

from contextlib import ExitStack

import ml_dtypes
import numpy as np

B, S, D, H = 2, 2048, 1024, 16
DK = D // H  # 64
N_CORES = 8
HG = H // (N_CORES // B)  # heads per core = 4
NPAIR = HG // 2  # head pairs per core = 2
MT = D // 128  # 8 m-tiles (contraction tiles for projections)
KT = S // 128  # 16 k-tiles
QC = S // 512  # 4 q-chunks

_CACHED = {}


def _build_program():
    import concourse.bass as bass
    import concourse.tile as tile
    from concourse import bacc, mybir

    BF16 = mybir.dt.bfloat16
    F32 = mybir.dt.float32
    AF = mybir.ActivationFunctionType

    nc = bacc.Bacc(trn_type="TRN2", target_bir_lowering=False, debug=False)

    # ---- DRAM I/O (per-core shapes) ----
    xqT = nc.dram_tensor("xqT", [D, S], BF16, kind="ExternalInput").ap()
    xkT = nc.dram_tensor("xkT", [D, S], BF16, kind="ExternalInput").ap()
    xvT = nc.dram_tensor("xvT", [D, S], BF16, kind="ExternalInput").ap()
    wqT = nc.dram_tensor("wqT", [D, 2 * 128], BF16, kind="ExternalInput").ap()
    wkT = nc.dram_tensor("wkT", [D, 2 * 128], BF16, kind="ExternalInput").ap()
    wvT = nc.dram_tensor("wvT", [D, 2 * 128], BF16, kind="ExternalInput").ap()
    woT = nc.dram_tensor("woT", [2 * 128, D], BF16, kind="ExternalInput").ap()
    attnT = nc.dram_tensor("attnT", [HG, S, S], BF16, kind="ExternalOutput").ap()
    outp = nc.dram_tensor("outp", [S, D], F32, kind="ExternalOutput").ap()

    scale = float(1.0 / np.sqrt(np.float32(DK)))

    with tile.TileContext(nc) as tc, ExitStack() as ctx:
        # preload the ACT table set containing BOTH Exp and Ln (and Copy) so
        # the auto-inserted table loads never ping-pong between sets.
        # act_func_set_id 6 = natural_log_exp_and_others (act_info.json).
        nc.scalar.add_instruction(
            mybir.InstLoadActFuncSet(
                name=nc.get_next_instruction_name(),
                ins=[],
                outs=[],
                act_func_set_id=6,
            )
        )

        # ---------- persistent pools ----------
        qk_pool = ctx.enter_context(tc.tile_pool(name="qk", bufs=4))
        vp_pool = ctx.enter_context(tc.tile_pool(name="vp", bufs=HG))
        wo_pool = ctx.enter_context(tc.tile_pool(name="wo", bufs=2))
        ctx_pool = ctx.enter_context(tc.tile_pool(name="ctxT", bufs=2))
        rz_pool = ctx.enter_context(tc.tile_pool(name="rz", bufs=3))
        sm_pool = ctx.enter_context(tc.tile_pool(name="sm", bufs=3))
        stage_pool = ctx.enter_context(tc.tile_pool(name="stage", bufs=3))
        osb_pool = ctx.enter_context(tc.tile_pool(name="osb", bufs=2))

        ps_small = ctx.enter_context(
            tc.tile_pool(name="ps_small", bufs=2, space="PSUM")
        )

        # persistent SBUF tensors
        QT2 = [qk_pool.tile([128, S], BF16, tag="qk", name=f"QT2_{p}") for p in range(NPAIR)]
        KT2 = [qk_pool.tile([128, S], BF16, tag="qk", name=f"KT2_{p}") for p in range(NPAIR)]
        VpT = [
            vp_pool.tile([128, KT * (DK + 1)], BF16, tag="vp", name=f"Vp_{h}")
            for h in range(HG)
        ]
        Vp = [
            [VpT[h][:, (DK + 1) * kt : (DK + 1) * (kt + 1)] for kt in range(KT)]
            for h in range(HG)
        ]
        woT_sb = [wo_pool.tile([128, D], BF16, tag="wo", name=f"woT_sb_{i}") for i in range(2)]
        ctxT2 = [ctx_pool.tile([128, S], BF16, tag="ctxT", name=f"ctxT2_{p}") for p in range(NPAIR)]

        for i in range(2):
            nc.sync.dma_start(woT_sb[i][:], woT[128 * i : 128 * (i + 1), :])

        def load_w_tiles(wT, nm, pool):
            tiles = []
            for mt in range(MT):
                t = pool.tile([128, 256], BF16, tag="wt", name=f"wt_{nm}_{mt}")
                nc.sync.dma_start(t[:], wT[128 * mt : 128 * (mt + 1), :])
                tiles.append(t)
            return tiles

        def load_x_tiles(xT, nm, pool):
            tiles = []
            for mt in range(MT):
                t = pool.tile([128, S], BF16, tag="xt", name=f"xt_{nm}_{mt}")
                nc.sync.dma_start(t[:], xT[128 * mt : 128 * (mt + 1), :])
                tiles.append(t)
            return tiles

        # ---------- phase 0: Q^T / K^T projections ----------
        with ExitStack() as p0:
            xt_pool = p0.enter_context(tc.tile_pool(name="xt", bufs=16))
            wt_pool = p0.enter_context(tc.tile_pool(name="wt", bufs=16))

            # weights first (small, needed by the first matmuls), then the
            # big input loads so DMA runs ahead
            wtq = load_w_tiles(wqT, "q", wt_pool)
            wtk = load_w_tiles(wkT, "k", wt_pool)
            xtq = load_x_tiles(xqT, "q", xt_pool)
            xtk = load_x_tiles(xkT, "k", xt_pool)

            # --- Q^T and K^T: out[do, s] = sum_m wT[m, do] * xT[m, s] ---
            for xt, wt, DEST in ((xtq, wtq, QT2), (xtk, wtk, KT2)):
                for p in range(NPAIR):
                    for c in range(QC):
                        ps = ps_small.tile([128, 512], F32, tag="ps_small")
                        for mt in range(MT):
                            nc.tensor.matmul(
                                ps[:],
                                wt[mt][:, 128 * p : 128 * (p + 1)],
                                xt[mt][:, 512 * c : 512 * (c + 1)],
                                start=(mt == 0),
                                stop=(mt == MT - 1),
                            )
                        nc.vector.tensor_copy(
                            DEST[p][:, 512 * c : 512 * (c + 1)], ps[:]
                        )

        def v_block(st):
            # one s-tile of the V projection: out[s, dv] = sum_m xvT * wvT
            ps = ps_small.tile([128, 256], F32, tag="ps_small", name=f"vps_{st}")
            for mt in range(MT):
                nc.tensor.matmul(
                    ps[:],
                    xtv[mt][:, 128 * st : 128 * (st + 1)],
                    wtv[mt][:],
                    start=(mt == 0),
                    stop=(mt == MT - 1),
                )
            for h in range(HG):
                nc.vector.tensor_copy(
                    Vp[h][st][:, 0:DK], ps[:, DK * h : DK * (h + 1)]
                )
                nc.vector.memset(Vp[h][st][:, DK : DK + 1], 1.0)

        # ---------- attention ----------
        with ExitStack() as pa:
            pt_pool_a = pa.enter_context(tc.tile_pool(name="pt_a", bufs=50))
            ps_s = pa.enter_context(tc.tile_pool(name="ps_s", bufs=3, space="PSUM"))
            pt_pools = [pt_pool_a]
            pv0 = ExitStack()
            xv_pool = pv0.enter_context(tc.tile_pool(name="xv", bufs=8))
            wv_pool = pv0.enter_context(tc.tile_pool(name="wv", bufs=8))
            wtv = load_w_tiles(wvT, "v", wv_pool)
            xtv = load_x_tiles(xvT, "v", xv_pool)

            # PT[h][kt][qh]: exp(S^T) bf16 half tiles [128 k, 1024 q]
            PT = [[[None] * 2 for _ in range(KT)] for _ in range(HG)]
            rzrep = {}  # (h, qh) -> [128, 1024] bf16 recipZ replicated

            def pv_job_a(p, hh, c):
                """PV accumulation for chunk c of head 2p+hh, plus Z ->
                recipZ on ACT.  Returns state for pv_job_b."""
                h = 2 * p + hh
                qh, sub = divmod(c, 2)
                pv = ps_small.tile(
                    [DK + 1, 512], F32, tag="ps_small", name=f"pv_{h}_{c}"
                )
                for kt in range(KT):
                    nc.tensor.matmul(
                        pv[:],
                        Vp[h][kt][:],
                        PT[h][kt][qh][:, 512 * sub : 512 * (sub + 1)],
                        start=(kt == 0),
                        stop=(kt == KT - 1),
                    )
                lnz = sm_pool.tile([1, 512], F32, tag="sm", name=f"lnz_{h}_{c}")
                nc.scalar.activation(lnz[:], pv[DK : DK + 1, :], AF.Ln)
                rz = sm_pool.tile([1, 512], F32, tag="sm", name=f"rz_{h}_{c}")
                nc.scalar.activation(rz[:], lnz[:], AF.Exp, scale=-1.0)
                return (p, hh, c, pv, rz)

            def pv_job_b(state):
                """Broadcast recipZ across partitions, normalize ctx^T,
                build bf16 recipZ replica for the attention staging."""
                p, hh, c, pv, rz = state
                h = 2 * p + hh
                qh, sub = divmod(c, 2)
                zrf = sm_pool.tile([128, 512], F32, tag="zrf", name=f"zrf_{h}_{c}")
                nc.gpsimd.partition_broadcast(zrf[:], rz[:])
                nc.vector.tensor_mul(
                    ctxT2[p][64 * hh : 64 * (hh + 1), 512 * c : 512 * (c + 1)],
                    pv[0:DK, :],
                    zrf[0:DK, :],
                )
                key = (h, qh)
                if key not in rzrep:
                    rzrep[key] = rz_pool.tile(
                        [128, 1024], BF16, tag="rz", name=f"rzrep_{h}_{qh}"
                    )
                nc.vector.tensor_copy(
                    rzrep[key][:, 512 * sub : 512 * (sub + 1)], zrf[:]
                )

            def attn_stage(h, qh):
                """Normalize P~^T and DMA the staged transposed attention.
                Alternate DVE / GpSimd so the two elementwise engines split
                the multiply work."""
                for kt in range(KT):
                    at = stage_pool.tile(
                        [128, 1024], BF16, tag="stage", name=f"at_{h}_{kt}_{qh}"
                    )
                    nc.vector.tensor_mul(at[:], PT[h][kt][qh][:], rzrep[(h, qh)][:])
                    nc.sync.dma_start(
                        attnT[
                            h,
                            128 * kt : 128 * (kt + 1),
                            1024 * qh : 1024 * (qh + 1),
                        ],
                        at[:],
                    )

            def scores_phase(p, qh, jobs, fillers=()):
                """S^T + exp for both heads of pair p over q-half qh, with
                the previous q-half's PV/normalize work interleaved.

                jobs: list of (p_prev, hh, c) chunk descriptors.
                fillers: callables emitted (spread out) into the phase."""
                pending_b = None
                points = {1: 0, 4: 1, 7: 2, 10: 3, 13: 4}
                fillers = list(fillers)
                nfill = len(fillers)
                for kt in range(KT):
                    if nfill:
                        lo = nfill * kt // KT
                        hi = nfill * (kt + 1) // KT
                        for fi in range(lo, hi):
                            fillers[fi]()
                    pss = []
                    for hh in range(2):
                        h = 2 * p + hh
                        pool = pt_pools[(2 * kt + hh) % len(pt_pools)]
                        PT[h][kt][qh] = pool.tile(
                            [128, 1024], BF16, tag="pt", name=f"PT_{h}_{kt}_{qh}"
                        )
                        pss.append(
                            ps_s.tile(
                                [128, 1024], F32, tag="ps_s", name=f"st_{h}_{kt}_{qh}"
                            )
                        )
                    # interleave the two heads' matmuls so consecutive PE
                    # instructions target different row groups (0-63 / 64-127)
                    # and run concurrently on the split PE array.
                    for sub in range(2):
                        q0 = 1024 * qh + 512 * sub
                        for hh in range(2):
                            nc.tensor.matmul(
                                pss[hh][:, 512 * sub : 512 * (sub + 1)],
                                KT2[p][
                                    64 * hh : 64 * (hh + 1),
                                    128 * kt : 128 * (kt + 1),
                                ],
                                QT2[p][64 * hh : 64 * (hh + 1), q0 : q0 + 512],
                                start=True,
                                stop=True,
                            )
                    for hh in range(2):
                        h = 2 * p + hh
                        nc.scalar.activation(
                            PT[h][kt][qh][:], pss[hh][:], AF.Exp, scale=scale
                        )
                    if kt in points:
                        j = points[kt]
                        if pending_b is not None:
                            pv_job_b(pending_b)
                            jp, jhh, jc = jobs[j - 1]
                            if jc % 2 == 1:  # second chunk of this head done
                                attn_stage(2 * jp + jhh, jc // 2)
                            pending_b = None
                        if j < len(jobs):
                            pending_b = pv_job_a(*jobs[j])
                # drain leftovers (the j=4 point only handles job 3's b-half)
                if pending_b is not None:
                    pv_job_b(pending_b)
                    jp, jhh, jc = jobs[-1]
                    if jc % 2 == 1:
                        attn_stage(2 * jp + jhh, jc // 2)

            def jobs_for(p, qh):
                return [
                    (p, 0, 2 * qh + 0),
                    (p, 0, 2 * qh + 1),
                    (p, 1, 2 * qh + 0),
                    (p, 1, 2 * qh + 1),
                ]

            def outproj(qts):
                for qt in qts:
                    for mc in range(2):
                        ps = ps_small.tile(
                            [128, 512], F32, tag="ps_small", name=f"ops_{qt}_{mc}"
                        )
                        for p in range(NPAIR):
                            nc.tensor.matmul(
                                ps[:],
                                ctxT2[p][:, 128 * qt : 128 * (qt + 1)],
                                woT_sb[p][:, 512 * mc : 512 * (mc + 1)],
                                start=(p == 0),
                                stop=(p == NPAIR - 1),
                            )
                        osb = osb_pool.tile(
                            [128, 512], F32, tag="osb", name=f"osb_{qt}_{mc}"
                        )
                        nc.scalar.copy(osb[:], ps[:])
                        nc.sync.dma_start(
                            outp[
                                128 * qt : 128 * (qt + 1),
                                512 * mc : 512 * (mc + 1),
                            ],
                            osb[:],
                        )

            import functools

            scores_phase(
                0, 0, [], fillers=[functools.partial(v_block, st) for st in range(KT)]
            )
            pv0.close()  # xv/wv freed
            scores_phase(0, 1, jobs_for(0, 0))
            scores_phase(1, 0, jobs_for(0, 1))
            scores_phase(1, 1, jobs_for(1, 0))
            # drain: PV for the last q-half, interleaved with the output
            # projection for the q-tiles whose ctx columns are already final
            # (chunks 0-1 = q-tiles 0-7 completed during the last phase).
            jobs = jobs_for(1, 1)
            st0 = pv_job_a(*jobs[0])
            st1 = pv_job_a(*jobs[1])
            pv_job_b(st0)
            outproj(range(0, 3))
            pv_job_b(st1)
            st2 = pv_job_a(*jobs[2])
            attn_stage(2 * jobs[1][0] + jobs[1][1], 1)
            outproj(range(3, 6))
            st3 = pv_job_a(*jobs[3])
            pv_job_b(st2)
            outproj(range(6, 8))
            pv_job_b(st3)
            attn_stage(2 * jobs[3][0] + jobs[3][1], 1)
            outproj(range(8, KT))

    nc.compile()
    return nc


def _get_program():
    if "nc" not in _CACHED:
        _CACHED["nc"] = _build_program()
    return _CACHED["nc"]


def _reference_numpy(query, key, value, mask, w_q, w_k, w_v, w_o, b_o):
    """Fallback for inputs the fast path does not handle (masked rows)."""
    scale = np.sqrt(np.float32(DK))
    q = np.asarray(query, np.float32)
    k = np.asarray(key, np.float32)
    v = np.asarray(value, np.float32)
    Q = (q @ np.asarray(w_q, np.float32).T).reshape(B, S, H, DK).transpose(0, 2, 1, 3)
    K = (k @ np.asarray(w_k, np.float32).T).reshape(B, S, H, DK).transpose(0, 2, 1, 3)
    V = (v @ np.asarray(w_v, np.float32).T).reshape(B, S, H, DK).transpose(0, 2, 1, 3)
    scores = np.einsum("bhqd,bhkd->bhqk", Q, K) / scale
    m = np.asarray(mask)[:, None, None, :]
    scores = np.where(m == 0, np.float32(-1e9), scores)
    scores -= scores.max(-1, keepdims=True)
    e = np.exp(scores)
    attn = (e / e.sum(-1, keepdims=True)).astype(np.float32)
    ctx = np.einsum("bhqk,bhkd->bhqd", attn, V)
    ctx = ctx.transpose(0, 2, 1, 3).reshape(B, S, D)
    out = (ctx @ np.asarray(w_o, np.float32).T + np.asarray(b_o, np.float32)).astype(
        np.float32
    )
    return out, attn


def kernel(query, key, value, mask, w_q, w_k, w_v, w_o, b_o):
    from concourse.bass_utils import run_bass_kernel_spmd

    query = np.asarray(query, np.float32)
    key = np.asarray(key, np.float32)
    value = np.asarray(value, np.float32)
    mask_np = np.asarray(mask)
    if not np.all(mask_np == 1):
        return _reference_numpy(query, key, value, mask_np, w_q, w_k, w_v, w_o, b_o)
    w_q = np.asarray(w_q, np.float32)
    w_k = np.asarray(w_k, np.float32)
    w_v = np.asarray(w_v, np.float32)
    w_o = np.asarray(w_o, np.float32)
    b_o = np.asarray(b_o, np.float32)

    nc = _get_program()

    bf = ml_dtypes.bfloat16
    in_maps = []
    for c in range(N_CORES):
        b, g = divmod(c, N_CORES // B)
        hs = slice(256 * g, 256 * (g + 1))
        in_maps.append(
            {
                "xqT": np.ascontiguousarray(query[b].T).astype(bf),
                "xkT": np.ascontiguousarray(key[b].T).astype(bf),
                "xvT": np.ascontiguousarray(value[b].T).astype(bf),
                "wqT": np.ascontiguousarray(w_q[hs, :].T).astype(bf),
                "wkT": np.ascontiguousarray(w_k[hs, :].T).astype(bf),
                "wvT": np.ascontiguousarray(w_v[hs, :].T).astype(bf),
                "woT": np.ascontiguousarray(w_o[:, hs].T).astype(bf),
            }
        )

    trace = bool(int(os.environ.get("TRN_TRACE", "0")))
    res = run_bass_kernel_spmd(nc, in_maps, list(range(N_CORES)), trace=trace)
    if trace and res.exec_time_ns is not None:
        print(f"HW exec time: {res.exec_time_ns} ns")
        _CACHED["exec_time_ns"] = res.exec_time_ns
        _CACHED["trace"] = res.instructions_and_trace

    out = np.empty((B, S, D), np.float32)
    attn = np.empty((B, H, S, S), np.float32)
    partials = np.zeros((B, S, D), np.float32)
    for c in range(N_CORES):
        b, g = divmod(c, N_CORES // B)
        r = res.results[c]
        partials[b] += r["outp"]
        a = r["attnT"]  # [HG, S(k), S(q)] bf16
        for j in range(HG):
            attn[b, HG * g + j] = a[j].T.astype(np.float32)
    for b in range(B):
        out[b] = partials[b] + b_o
    return out, attn


# revision 19
# speedup vs baseline: 1.1414x; 1.0414x over previous
"""MultiHeadAttention forward on 8 trn2 NeuronCores (Bass/Tile).

Problem: B=2, S=2048, D=1024, H=16, DK=64.  reference returns (out, attn):
  Q = q @ Wq.T ; K = k @ Wk.T ; V = v @ Wv.T   (per-head split)
  S = Q K^T / sqrt(DK) ; A = softmax(S) ; ctx = A V ; out = ctx @ Wo.T + b_o

Sharding: data-parallel over B (2) x tensor-parallel over heads (4 groups of
4 heads) = 8 cores.  Each core computes its 4 heads' attention entirely in the
transposed orientation (S^T with k on partitions), which gives softmax row
sums for free via a ones-column appended to V during the PV matmul, and needs
only ONE exp pass on the scalar engine.  The normalized attention is staged
TRANSPOSED ([head, k, q]) in bf16; the host transposes/upcasts during the
unshard step.  Output projection partial products (one per head group) are
summed on the host during the gather.

All GEMMs run in bf16 with fp32 PSUM accumulation.  Scores matmuls pack the
two heads of a pair into disjoint PE row groups (K=64 each) so they run
concurrently.  PV/attention work for one q-half is interleaved into the next
q-half's scores stream to keep the PE dense.
"""

import os
import sys

sys.path.insert(0, "/opt/trn_rl_repo")

from contextlib import ExitStack

import ml_dtypes
import numpy as np

B, S, D, H = 2, 2048, 1024, 16
DK = D // H  # 64
N_CORES = 8
HG = H // (N_CORES // B)  # heads per core = 4
NPAIR = HG // 2  # head pairs per core = 2
MT = D // 128  # 8 m-tiles (contraction tiles for projections)
KT = S // 128  # 16 k-tiles
QC = S // 512  # 4 q-chunks

_CACHED = {}


def _build_program():
    import concourse.bass as bass
    import concourse.tile as tile
    from concourse import bacc, mybir

    BF16 = mybir.dt.bfloat16
    F32 = mybir.dt.float32
    AF = mybir.ActivationFunctionType

    nc = bacc.Bacc(trn_type="TRN2", target_bir_lowering=False, debug=False)

    # ---- DRAM I/O (per-core shapes) ----
    xqT = nc.dram_tensor("xqT", [D, S], BF16, kind="ExternalInput").ap()
    xkT = nc.dram_tensor("xkT", [D, S], BF16, kind="ExternalInput").ap()
    xvT = nc.dram_tensor("xvT", [D, S], BF16, kind="ExternalInput").ap()
    wqT = nc.dram_tensor("wqT", [D, 2 * 128], BF16, kind="ExternalInput").ap()
    wkT = nc.dram_tensor("wkT", [D, 2 * 128], BF16, kind="ExternalInput").ap()
    wvT = nc.dram_tensor("wvT", [D, 2 * 128], BF16, kind="ExternalInput").ap()
    woT = nc.dram_tensor("woT", [2 * 128, D], BF16, kind="ExternalInput").ap()
    attnT = nc.dram_tensor("attnT", [HG, S, S], BF16, kind="ExternalOutput").ap()
    outp = nc.dram_tensor("outp", [S, D], F32, kind="ExternalOutput").ap()

    scale = float(1.0 / np.sqrt(np.float32(DK)))

    with tile.TileContext(nc) as tc, ExitStack() as ctx:
        # preload the ACT table set containing BOTH Exp and Ln (and Copy) so
        # the auto-inserted table loads never ping-pong between sets.
        # act_func_set_id 6 = natural_log_exp_and_others (act_info.json).
        nc.scalar.add_instruction(
            mybir.InstLoadActFuncSet(
                name=nc.get_next_instruction_name(),
                ins=[],
                outs=[],
                act_func_set_id=6,
            )
        )

        # ---------- persistent pools ----------
        qk_pool = ctx.enter_context(tc.tile_pool(name="qk", bufs=4))
        vp_pool = ctx.enter_context(tc.tile_pool(name="vp", bufs=HG))
        wo_pool = ctx.enter_context(tc.tile_pool(name="wo", bufs=2))
        ctx_pool = ctx.enter_context(tc.tile_pool(name="ctxT", bufs=2))
        rz_pool = ctx.enter_context(tc.tile_pool(name="rz", bufs=3))
        sm_pool = ctx.enter_context(tc.tile_pool(name="sm", bufs=3))
        stage_pool = ctx.enter_context(tc.tile_pool(name="stage", bufs=3))
        osb_pool = ctx.enter_context(tc.tile_pool(name="osb", bufs=2))

        ps_small = ctx.enter_context(
            tc.tile_pool(name="ps_small", bufs=2, space="PSUM")
        )

        # persistent SBUF tensors
        QT2 = [qk_pool.tile([128, S], BF16, tag="qk", name=f"QT2_{p}") for p in range(NPAIR)]
        KT2 = [qk_pool.tile([128, S], BF16, tag="qk", name=f"KT2_{p}") for p in range(NPAIR)]
        VpT = [
            vp_pool.tile([128, KT * (DK + 1)], BF16, tag="vp", name=f"Vp_{h}")
            for h in range(HG)
        ]
        Vp = [
            [VpT[h][:, (DK + 1) * kt : (DK + 1) * (kt + 1)] for kt in range(KT)]
            for h in range(HG)
        ]
        woT_sb = [wo_pool.tile([128, D], BF16, tag="wo", name=f"woT_sb_{i}") for i in range(2)]
        ctxT2 = [ctx_pool.tile([128, S], BF16, tag="ctxT", name=f"ctxT2_{p}") for p in range(NPAIR)]

        for i in range(2):
            nc.sync.dma_start(woT_sb[i][:], woT[128 * i : 128 * (i + 1), :])

        def load_w_tiles(wT, nm, pool):
            tiles = []
            for mt in range(MT):
                t = pool.tile([128, 256], BF16, tag="wt", name=f"wt_{nm}_{mt}")
                nc.sync.dma_start(t[:], wT[128 * mt : 128 * (mt + 1), :])
                tiles.append(t)
            return tiles

        def load_x_tiles(xT, nm, pool):
            tiles = []
            for mt in range(MT):
                t = pool.tile([128, S], BF16, tag="xt", name=f"xt_{nm}_{mt}")
                nc.sync.dma_start(t[:], xT[128 * mt : 128 * (mt + 1), :])
                tiles.append(t)
            return tiles

        def load_x_half_tiles(xT, nm, pool):
            # [q-half][mt] half tiles so the first projection chunk only
            # waits on 2 MB instead of the full 4 MB tensor
            halves = []
            for hf in range(2):
                tiles = []
                for mt in range(MT):
                    t = pool.tile(
                        [128, S // 2], BF16, tag="xth", name=f"xth_{nm}_{mt}_{hf}"
                    )
                    nc.sync.dma_start(
                        t[:],
                        xT[128 * mt : 128 * (mt + 1), 1024 * hf : 1024 * (hf + 1)],
                    )
                    tiles.append(t)
                halves.append(tiles)
            return halves

        # ---------- phase 0: Q^T / K^T projections ----------
        with ExitStack() as p0:
            xt_pool = p0.enter_context(tc.tile_pool(name="xt", bufs=32))
            wt_pool = p0.enter_context(tc.tile_pool(name="wt", bufs=16))

            # weights first (small, needed by the first matmuls), then the
            # big input loads so DMA runs ahead
            wtq = load_w_tiles(wqT, "q", wt_pool)
            wtk = load_w_tiles(wkT, "k", wt_pool)
            xtq = load_x_half_tiles(xqT, "q", xt_pool)
            xtk = load_x_half_tiles(xkT, "k", xt_pool)

            # --- Q^T and K^T: out[do, s] = sum_m wT[m, do] * xT[m, s] ---
            for xt, wt, DEST in ((xtq, wtq, QT2), (xtk, wtk, KT2)):
                for p in range(NPAIR):
                    for c in range(QC):
                        ps = ps_small.tile([128, 512], F32, tag="ps_small")
                        for mt in range(MT):
                            nc.tensor.matmul(
                                ps[:],
                                wt[mt][:, 128 * p : 128 * (p + 1)],
                                xt[c // 2][mt][
                                    :, 512 * (c % 2) : 512 * (c % 2 + 1)
                                ],
                                start=(mt == 0),
                                stop=(mt == MT - 1),
                            )
                        nc.vector.tensor_copy(
                            DEST[p][:, 512 * c : 512 * (c + 1)], ps[:]
                        )

        def v_block(st):
            # one s-tile of the V projection: out[s, dv] = sum_m xvT * wvT
            ps = ps_small.tile([128, 256], F32, tag="ps_small", name=f"vps_{st}")
            for mt in range(MT):
                nc.tensor.matmul(
                    ps[:],
                    xtv[mt][:, 128 * st : 128 * (st + 1)],
                    wtv[mt][:],
                    start=(mt == 0),
                    stop=(mt == MT - 1),
                )
            for h in range(HG):
                nc.vector.tensor_copy(
                    Vp[h][st][:, 0:DK], ps[:, DK * h : DK * (h + 1)]
                )
                nc.vector.memset(Vp[h][st][:, DK : DK + 1], 1.0)

        # ---------- attention ----------
        with ExitStack() as pa:
            pt_pool_a = pa.enter_context(tc.tile_pool(name="pt_a", bufs=50))
            ps_s = pa.enter_context(tc.tile_pool(name="ps_s", bufs=3, space="PSUM"))
            pt_pools = [pt_pool_a]
            pv0 = ExitStack()
            xv_pool = pv0.enter_context(tc.tile_pool(name="xv", bufs=8))
            wv_pool = pv0.enter_context(tc.tile_pool(name="wv", bufs=8))
            wtv = load_w_tiles(wvT, "v", wv_pool)
            xtv = load_x_tiles(xvT, "v", xv_pool)

            # PT[h][kt][qh]: exp(S^T) bf16 half tiles [128 k, 1024 q]
            PT = [[[None] * 2 for _ in range(KT)] for _ in range(HG)]
            rzrep = {}  # (h, qh) -> [128, 1024] bf16 recipZ replicated

            def pv_job_a(p, hh, c):
                """PV accumulation for chunk c of head 2p+hh, plus Z ->
                recipZ on ACT.  Returns state for pv_job_b."""
                h = 2 * p + hh
                qh, sub = divmod(c, 2)
                pv = ps_small.tile(
                    [DK + 1, 512], F32, tag="ps_small", name=f"pv_{h}_{c}"
                )
                for kt in range(KT):
                    nc.tensor.matmul(
                        pv[:],
                        Vp[h][kt][:],
                        PT[h][kt][qh][:, 512 * sub : 512 * (sub + 1)],
                        start=(kt == 0),
                        stop=(kt == KT - 1),
                    )
                lnz = sm_pool.tile([1, 512], F32, tag="sm", name=f"lnz_{h}_{c}")
                nc.scalar.activation(lnz[:], pv[DK : DK + 1, :], AF.Ln)
                rz = sm_pool.tile([1, 512], F32, tag="sm", name=f"rz_{h}_{c}")
                nc.scalar.activation(rz[:], lnz[:], AF.Exp, scale=-1.0)
                return (p, hh, c, pv, rz)

            def pv_job_b(state):
                """Broadcast recipZ across partitions, normalize ctx^T,
                build bf16 recipZ replica for the attention staging."""
                p, hh, c, pv, rz = state
                h = 2 * p + hh
                qh, sub = divmod(c, 2)
                zrf = sm_pool.tile([128, 512], F32, tag="zrf", name=f"zrf_{h}_{c}")
                nc.gpsimd.partition_broadcast(zrf[:], rz[:])
                nc.vector.tensor_mul(
                    ctxT2[p][64 * hh : 64 * (hh + 1), 512 * c : 512 * (c + 1)],
                    pv[0:DK, :],
                    zrf[0:DK, :],
                )
                key = (h, qh)
                if key not in rzrep:
                    rzrep[key] = rz_pool.tile(
                        [128, 1024], BF16, tag="rz", name=f"rzrep_{h}_{qh}"
                    )
                nc.vector.tensor_copy(
                    rzrep[key][:, 512 * sub : 512 * (sub + 1)], zrf[:]
                )

            def attn_stage(h, qh):
                """Normalize P~^T and DMA the staged transposed attention.
                Alternate DVE / GpSimd so the two elementwise engines split
                the multiply work."""
                for kt in range(KT):
                    at = stage_pool.tile(
                        [128, 1024], BF16, tag="stage", name=f"at_{h}_{kt}_{qh}"
                    )
                    nc.vector.tensor_mul(at[:], PT[h][kt][qh][:], rzrep[(h, qh)][:])
                    nc.sync.dma_start(
                        attnT[
                            h,
                            128 * kt : 128 * (kt + 1),
                            1024 * qh : 1024 * (qh + 1),
                        ],
                        at[:],
                    )

            def scores_phase(p, qh, jobs, fillers=()):
                """S^T + exp for both heads of pair p over q-half qh, with
                the previous q-half's PV/normalize work interleaved.

                jobs: list of (p_prev, hh, c) chunk descriptors.
                fillers: callables emitted (spread out) into the phase."""
                pending_b = None
                points = {1: 0, 4: 1, 7: 2, 10: 3, 13: 4}
                fillers = list(fillers)
                nfill = len(fillers)
                for kt in range(KT):
                    if nfill:
                        lo = nfill * kt // KT
                        hi = nfill * (kt + 1) // KT
                        for fi in range(lo, hi):
                            fillers[fi]()
                    pss = []
                    for hh in range(2):
                        h = 2 * p + hh
                        pool = pt_pools[(2 * kt + hh) % len(pt_pools)]
                        PT[h][kt][qh] = pool.tile(
                            [128, 1024], BF16, tag="pt", name=f"PT_{h}_{kt}_{qh}"
                        )
                        pss.append(
                            ps_s.tile(
                                [128, 1024], F32, tag="ps_s", name=f"st_{h}_{kt}_{qh}"
                            )
                        )
                    # interleave the two heads' matmuls so consecutive PE
                    # instructions target different row groups (0-63 / 64-127)
                    # and run concurrently on the split PE array.
                    for sub in range(2):
                        q0 = 1024 * qh + 512 * sub
                        for hh in range(2):
                            nc.tensor.matmul(
                                pss[hh][:, 512 * sub : 512 * (sub + 1)],
                                KT2[p][
                                    64 * hh : 64 * (hh + 1),
                                    128 * kt : 128 * (kt + 1),
                                ],
                                QT2[p][64 * hh : 64 * (hh + 1), q0 : q0 + 512],
                                start=True,
                                stop=True,
                            )
                    for hh in range(2):
                        h = 2 * p + hh
                        nc.scalar.activation(
                            PT[h][kt][qh][:], pss[hh][:], AF.Exp, scale=scale
                        )
                    if kt in points:
                        j = points[kt]
                        if pending_b is not None:
                            pv_job_b(pending_b)
                            jp, jhh, jc = jobs[j - 1]
                            if jc % 2 == 1:  # second chunk of this head done
                                attn_stage(2 * jp + jhh, jc // 2)
                            pending_b = None
                        if j < len(jobs):
                            pending_b = pv_job_a(*jobs[j])
                # drain leftovers (the j=4 point only handles job 3's b-half)
                if pending_b is not None:
                    pv_job_b(pending_b)
                    jp, jhh, jc = jobs[-1]
                    if jc % 2 == 1:
                        attn_stage(2 * jp + jhh, jc // 2)

            def jobs_for(p, qh):
                return [
                    (p, 0, 2 * qh + 0),
                    (p, 0, 2 * qh + 1),
                    (p, 1, 2 * qh + 0),
                    (p, 1, 2 * qh + 1),
                ]

            def outproj(qts):
                for qt in qts:
                    for mc in range(2):
                        ps = ps_small.tile(
                            [128, 512], F32, tag="ps_small", name=f"ops_{qt}_{mc}"
                        )
                        for p in range(NPAIR):
                            nc.tensor.matmul(
                                ps[:],
                                ctxT2[p][:, 128 * qt : 128 * (qt + 1)],
                                woT_sb[p][:, 512 * mc : 512 * (mc + 1)],
                                start=(p == 0),
                                stop=(p == NPAIR - 1),
                            )
                        osb = osb_pool.tile(
                            [128, 512], F32, tag="osb", name=f"osb_{qt}_{mc}"
                        )
                        nc.scalar.copy(osb[:], ps[:])
                        nc.sync.dma_start(
                            outp[
                                128 * qt : 128 * (qt + 1),
                                512 * mc : 512 * (mc + 1),
                            ],
                            osb[:],
                        )

            import functools

            scores_phase(
                0, 0, [], fillers=[functools.partial(v_block, st) for st in range(KT)]
            )
            pv0.close()  # xv/wv freed
            scores_phase(0, 1, jobs_for(0, 0))
            scores_phase(1, 0, jobs_for(0, 1))
            scores_phase(1, 1, jobs_for(1, 0))
            # drain: PV for the last q-half, interleaved with the output
            # projection for the q-tiles whose ctx columns are already final
            # (chunks 0-1 = q-tiles 0-7 completed during the last phase).
            jobs = jobs_for(1, 1)
            st0 = pv_job_a(*jobs[0])
            st1 = pv_job_a(*jobs[1])
            pv_job_b(st0)
            outproj(range(0, 3))
            pv_job_b(st1)
            st2 = pv_job_a(*jobs[2])
            attn_stage(2 * jobs[1][0] + jobs[1][1], 1)
            outproj(range(3, 6))
            st3 = pv_job_a(*jobs[3])
            pv_job_b(st2)
            outproj(range(6, 8))
            pv_job_b(st3)
            attn_stage(2 * jobs[3][0] + jobs[3][1], 1)
            outproj(range(8, KT))

    nc.compile()
    return nc


def _get_program():
    if "nc" not in _CACHED:
        _CACHED["nc"] = _build_program()
    return _CACHED["nc"]


def _reference_numpy(query, key, value, mask, w_q, w_k, w_v, w_o, b_o):
    """Fallback for inputs the fast path does not handle (masked rows)."""
    scale = np.sqrt(np.float32(DK))
    q = np.asarray(query, np.float32)
    k = np.asarray(key, np.float32)
    v = np.asarray(value, np.float32)
    Q = (q @ np.asarray(w_q, np.float32).T).reshape(B, S, H, DK).transpose(0, 2, 1, 3)
    K = (k @ np.asarray(w_k, np.float32).T).reshape(B, S, H, DK).transpose(0, 2, 1, 3)
    V = (v @ np.asarray(w_v, np.float32).T).reshape(B, S, H, DK).transpose(0, 2, 1, 3)
    scores = np.einsum("bhqd,bhkd->bhqk", Q, K) / scale
    m = np.asarray(mask)[:, None, None, :]
    scores = np.where(m == 0, np.float32(-1e9), scores)
    scores -= scores.max(-1, keepdims=True)
    e = np.exp(scores)
    attn = (e / e.sum(-1, keepdims=True)).astype(np.float32)
    ctx = np.einsum("bhqk,bhkd->bhqd", attn, V)
    ctx = ctx.transpose(0, 2, 1, 3).reshape(B, S, D)
    out = (ctx @ np.asarray(w_o, np.float32).T + np.asarray(b_o, np.float32)).astype(
        np.float32
    )
    return out, attn


def kernel(query, key, value, mask, w_q, w_k, w_v, w_o, b_o):
    from concourse.bass_utils import run_bass_kernel_spmd

    query = np.asarray(query, np.float32)
    key = np.asarray(key, np.float32)
    value = np.asarray(value, np.float32)
    mask_np = np.asarray(mask)
    if not np.all(mask_np == 1):
        return _reference_numpy(query, key, value, mask_np, w_q, w_k, w_v, w_o, b_o)
    w_q = np.asarray(w_q, np.float32)
    w_k = np.asarray(w_k, np.float32)
    w_v = np.asarray(w_v, np.float32)
    w_o = np.asarray(w_o, np.float32)
    b_o = np.asarray(b_o, np.float32)

    nc = _get_program()

    bf = ml_dtypes.bfloat16
    in_maps = []
    for c in range(N_CORES):
        b, g = divmod(c, N_CORES // B)
        hs = slice(256 * g, 256 * (g + 1))
        in_maps.append(
            {
                "xqT": np.ascontiguousarray(query[b].T).astype(bf),
                "xkT": np.ascontiguousarray(key[b].T).astype(bf),
                "xvT": np.ascontiguousarray(value[b].T).astype(bf),
                "wqT": np.ascontiguousarray(w_q[hs, :].T).astype(bf),
                "wkT": np.ascontiguousarray(w_k[hs, :].T).astype(bf),
                "wvT": np.ascontiguousarray(w_v[hs, :].T).astype(bf),
                "woT": np.ascontiguousarray(w_o[:, hs].T).astype(bf),
            }
        )

    trace = bool(int(os.environ.get("TRN_TRACE", "0")))
    res = run_bass_kernel_spmd(nc, in_maps, list(range(N_CORES)), trace=trace)
    if trace and res.exec_time_ns is not None:
        print(f"HW exec time: {res.exec_time_ns} ns")
        _CACHED["exec_time_ns"] = res.exec_time_ns
        _CACHED["trace"] = res.instructions_and_trace

    out = np.empty((B, S, D), np.float32)
    attn = np.empty((B, H, S, S), np.float32)
    partials = np.zeros((B, S, D), np.float32)
    for c in range(N_CORES):
        b, g = divmod(c, N_CORES // B)
        r = res.results[c]
        partials[b] += r["outp"]
        a = r["attnT"]  # [HG, S(k), S(q)] bf16
        for j in range(HG):
            attn[b, HG * g + j] = a[j].T.astype(np.float32)
    for b in range(B):
        out[b] = partials[b] + b_o
    return out, attn
